# revision 19
# baseline (speedup 1.0000x reference)
"""Additive-attention kernel for Trainium2 (8 NeuronCores, SPMD).

Problem (per batch b of B=4):
    xt      = x[b].T                                  # (N=512, D=96)
    g1      = xt @ Wg1.T                              # (512, 256)
    g2      = xt @ Wg2.T                              # (512, 256)
    score   = sum_a Wa[a] * tanh(g1[n,a] + g2[m,a] + bg[a])    # (512, 512)
    att     = sigmoid(score + Wa_b + ba)
    out[b]  = att @ xt                                # (512, 96)

Sharding: core c handles batch b = c//2 and query-rows n in
[(c%2)*256, (c%2)*256+256).  Each core computes its full out rows; the
host concatenates.

Algorithm (v2, Fourier factorization): approximate
    tanh(u+v) ~= sum_{j=1..FJ} BJ[j-1] * sin(j*S*(u+v)),   S = pi/FL
(coefficients from a smoothness-regularized weighted least-squares fit
of tanh on |u+v|<=12 with free periodic completion).  Each harmonic
separates:  sin(jTu+jTv) = sin(jTu)cos(jTv) + cos(jTu)sin(jTv), so the
whole N x N score matrix becomes plain matmuls over a contraction dim
of (a, j, sin|cos) pairs:

  - theta = S*(g + bg) per side via PE matmuls (K=D=96).
  - base features sin(theta), cos(theta) via ACT Sin (args stay within
    the LUT's [-pi, pi] domain: |S*g| + pi/2 < pi for |g| <= FL/2).
  - harmonics via the Chebyshev recurrence f_j = 2cos(theta)*f_{j-1} -
    f_{j-2} on the Vector engine in fp16 (2 tensor_tensor ops per j
    over a combined [128, 2, 1536] tile holding both sides and both
    sin/cos lanes).
  - u-side features scaled by Wa[a]*BJ[j-1] (tensor_scalar, per-
    partition Wa vector + immediate).
  - scoring: per (j, fn, a-chunk, m-block) matmul with the v-side
    feature block as the stationary operand -> scoreT[m, n] accumulates
    into 4 PSUM banks [128, 256] fp32.
  - sigmoid (+Wa_b+ba) PSUM->SBUF fp16 yields attT[m, n] directly, the
    lhsT of the final out[n, d] matmul against x[b].T (fp16).
"""

import numpy as np

B, D, N, A = 4, 96, 512, 256
NH = N // 2          # query rows per core
NCORES = 8

FJ = 13
FL = 13.0
FS = float(np.pi / FL)
BJ = [1.25339337, -0.01945643, 0.37026378, -0.0301986, 0.17909742,
      -0.02951455, 0.0971231, -0.02324765, 0.05208288, -0.01869683,
      0.02430917, -0.01051296, 0.01973076]

_cache = {}


def _build_nc_v2():
    import concourse.bacc as bacc
    import concourse.mybir as mybir
    from concourse import tile

    f32 = mybir.dt.float32
    f16 = mybir.dt.float16
    AF = mybir.ActivationFunctionType
    MULT = mybir.AluOpType.mult

    nc = bacc.Bacc("TRN2", target_bir_lowering=False)

    # packed inputs (fp32: the fp16 variant shifts SBUF tile addresses
    # into a layout that slows DVE tensor_tensor ops by ~20%)
    vin = nc.dram_tensor("vin", [D, A + N], f32, kind="ExternalInput")
    uin = nc.dram_tensor("uin", [D, A + NH], f32, kind="ExternalInput")
    biasv = nc.dram_tensor("biasv", [128, 7], f32, kind="ExternalInput")
    xkT = nc.dram_tensor("xkT", [N, D], f16, kind="ExternalInput")
    out = nc.dram_tensor("out", [NH, D], f32, kind="ExternalOutput")

    with tile.TileContext(nc) as tc:
        with (
            tc.tile_pool(name="consts", bufs=1) as consts,
            tc.tile_pool(name="ufeat", bufs=1) as ufeat,
            tc.tile_pool(name="uscal", bufs=1) as uscal,
            tc.tile_pool(name="tmpp", bufs=2) as tmpp,
            tc.tile_pool(name="gps", bufs=2, space="PSUM") as gps,
            tc.tile_pool(name="scps", bufs=1, space="PSUM") as scps,
            tc.tile_pool(name="fps", bufs=1, space="PSUM") as fps,
            tc.tile_pool(name="attp", bufs=1) as attp,
            tc.tile_pool(name="opool", bufs=1) as opool,
        ):
            vin_sb = consts.tile([D, A + N], f32, tag="vin")
            uin_sb = consts.tile([D, A + NH], f32, tag="uin")
            biasv_sb = consts.tile([128, 7], f32, tag="biasv")
            xkT_sb = consts.tile([128, 4, D], f16, tag="xkT")
            w2_sb = vin_sb[:, :A]
            xk_sb = vin_sb[:, A:A + N]
            w1_sb = uin_sb[:, :A]
            xq_sb = uin_sb[:, A:A + NH]
            bsin_sb = biasv_sb[:, 0:2]
            bcos_sb = biasv_sb[:, 2:4]
            wav_sb = biasv_sb[:, 4:6]
            sgb_sb = biasv_sb[:, 6:7]

            # dummy Sin on garbage to preload ACT table sets during DMAs
            dummy = consts.tile([128, 1], f32, tag="dummy")
            nc.gpsimd.memset(dummy[:], 0.0)
            nc.scalar.activation(dummy[:], dummy[:], AF.Sin)

            nc.sync.dma_start(vin_sb[:], vin.ap())
            nc.sync.dma_start(biasv_sb[:], biasv.ap())
            nc.sync.dma_start(uin_sb[:], uin.ap())
            nc.scalar.dma_start(
                xkT_sb[:], xkT.ap().rearrange("(mb p) d -> p mb d", p=128)
            )

            # combined feature tiles, j = 1..FJ:
            # [128, (sin|cos), v-part(c*512+m) | u-part(1024 + c*256+n)]
            FV = N * 2            # 1024: v-part width
            FT = FV + NH * 2      # 1536: total width
            cf = [ufeat.tile([128, 2, FT], f16, tag=f"cf{j}", name=f"cf{j}")
                  if j >= 1 else None for j in range(FJ + 1)]
            us = [uscal.tile([128, 2, NH * 2], f16, tag=f"us{j}", name=f"us{j}")
                  if j >= 1 else None for j in range(FJ + 1)]
            twoc = consts.tile([128, 2, FT], f16, tag="twoc")

            # theta tiles + base features (j=1); v-side first, cos first
            thvs = []
            for c in range(2):
                thv = gps.tile([128, N], f32, tag="th", name=f"thv{c}")
                nc.tensor.matmul(thv[:], w2_sb[:, c * 128:(c + 1) * 128],
                                 xk_sb[:])
                thvs.append(thv)
            thus = []
            for c in range(2):
                thu = gps.tile([128, N], f32, tag="th", name=f"thu{c}")
                nc.tensor.matmul(thu[:, :NH], w1_sb[:, c * 128:(c + 1) * 128],
                                 xq_sb[:])
                thus.append(thu)
            for c in range(2):
                nc.scalar.activation(cf[1][:, 1, c * N:(c + 1) * N],
                                     thvs[c][:], AF.Sin,
                                     bias=bcos_sb[:, c:c + 1])
                nc.scalar.activation(cf[1][:, 0, c * N:(c + 1) * N],
                                     thvs[c][:], AF.Sin,
                                     bias=bsin_sb[:, c:c + 1])
            for c in range(2):
                nc.scalar.activation(cf[1][:, 1, FV + c * NH:FV + (c + 1) * NH],
                                     thus[c][:, :NH], AF.Sin,
                                     bias=bcos_sb[:, c:c + 1])
                nc.scalar.activation(cf[1][:, 0, FV + c * NH:FV + (c + 1) * NH],
                                     thus[c][:, :NH], AF.Sin,
                                     bias=bsin_sb[:, c:c + 1])

            for fn in range(2):
                nc.vector.tensor_scalar_mul(twoc[:, fn, :], cf[1][:, 1, :], 2.0)

            sc = [scps.tile([128, NH], f32, tag=f"sc{mb}", name=f"sc{mb}")
                  for mb in range(4)]

            for j in range(1, FJ + 1):
                if j == 2:
                    # f_2 = 2c*f_1 - f_0 with f_0 = (0, 1)
                    tmpc = tmpp.tile([128, 2, FT], f16, tag="tmpc")
                    nc.vector.tensor_mul(tmpc[:], cf[1][:], twoc[:])
                    nc.vector.tensor_copy(cf[2][:, 0, :], tmpc[:, 0, :])
                    nc.vector.tensor_scalar_add(cf[2][:, 1, :], tmpc[:, 1, :],
                                                -1.0)
                elif j >= 3:
                    tmpc = tmpp.tile([128, 2, FT], f16, tag="tmpc")
                    nc.vector.tensor_mul(tmpc[:], cf[j - 1][:], twoc[:])
                    nc.vector.tensor_sub(cf[j][:], tmpc[:], cf[j - 2][:])
                # scale u-part by Wa[a]*BJ[j-1] (both fn halves per op)
                for c in range(2):
                    nc.vector.tensor_scalar(
                        us[j][:, :, c * NH:(c + 1) * NH],
                        cf[j][:, :, FV + c * NH:FV + (c + 1) * NH],
                        wav_sb[:, c:c + 1], float(BJ[j - 1]),
                        MULT, MULT,
                    )
                # scoring: sin_u pairs cos_v, cos_u pairs sin_v
                for fn in range(2):
                    for c in range(2):
                        for mb in range(4):
                            nc.tensor.matmul(
                                sc[mb][:],
                                cf[j][:, 1 - fn,
                                      c * N + mb * 128: c * N + (mb + 1) * 128],
                                us[j][:, fn, c * NH:(c + 1) * NH],
                                start=(j == 1 and fn == 0 and c == 0),
                                stop=(j == FJ and fn == 1 and c == 1),
                                skip_group_check=True,
                            )

            attT = attp.tile([128, 4, NH], f16, tag="attT")
            out_sb = opool.tile([128, 2, D], f32, tag="out")
            fos = [fps.tile([128, D], f32, tag=f"fo{nb}", name=f"fo{nb}")
                   for nb in range(2)]
            for mb in range(4):
                nc.scalar.activation(
                    attT[:, mb, :], sc[mb][:], AF.Sigmoid, bias=sgb_sb[:, 0:1]
                )
                for nb in range(2):
                    nc.tensor.matmul(
                        fos[nb][:],
                        attT[:, mb, nb * 128:(nb + 1) * 128],
                        xkT_sb[:, mb, :],
                        start=(mb == 0),
                        stop=(mb == 3),
                        skip_group_check=True,
                    )
            for nb in range(2):
                nc.vector.tensor_copy(out_sb[:, nb, :], fos[nb][:])

            nc.sync.dma_start(
                out.ap().rearrange("(nb p) d -> p nb d", p=128), out_sb[:]
            )

    nc.compile()
    return nc


def _prep_inputs_v2(x, Wg1, Wg2, bg, Wa_w, Wa_b, ba):
    """Host-side packing/slicing only (no reference math)."""
    x = np.asarray(x, np.float32)
    w1s = FS * np.asarray(Wg1, np.float32).T
    w2s = FS * np.asarray(Wg2, np.float32).T
    bgv = FS * np.asarray(bg, np.float32)
    biasv = np.empty((128, 7), np.float32)
    biasv[:, 0:2] = bgv.reshape(2, 128).T
    biasv[:, 2:4] = bgv.reshape(2, 128).T + np.float32(np.pi / 2)
    biasv[:, 4:6] = np.asarray(Wa_w, np.float32).reshape(2, 128).T
    biasv[:, 6] = float(np.asarray(Wa_b).ravel()[0]) \
        + float(np.asarray(ba).ravel()[0])
    in_maps = []
    for c in range(NCORES):
        b, half = c // 2, c % 2
        xb = x[b]
        vin = np.ascontiguousarray(np.concatenate([w2s, xb], axis=1),
                                   dtype=np.float32)
        uin = np.ascontiguousarray(
            np.concatenate([w1s, xb[:, half * NH:(half + 1) * NH]], axis=1),
            dtype=np.float32)
        in_maps.append({
            "vin": vin,
            "uin": uin,
            "biasv": np.ascontiguousarray(biasv),
            "xkT": np.ascontiguousarray(xb.T.astype(np.float16)),
        })
    return in_maps


def _run(inputs, trace=False):
    from concourse.bass_utils import run_bass_kernel_spmd

    if "nc" not in _cache:
        _cache["nc"] = _build_nc_v2()
    nc = _cache["nc"]
    in_maps = _prep_inputs_v2(**inputs)
    res = run_bass_kernel_spmd(
        nc, in_maps, core_ids=list(range(NCORES)), trace=trace
    )
    out = np.empty((B, N, D), np.float32)
    for c in range(NCORES):
        b, half = c // 2, c % 2
        out[b, half * NH:(half + 1) * NH] = res.results[c]["out"]
    return out, res


def kernel(**inputs):
    out, _ = _run(inputs, trace=False)
    return out


# revision 20
# speedup vs baseline: 1.0712x; 1.0712x over previous
"""Additive-attention kernel for Trainium2 (8 NeuronCores, SPMD).

Problem (per batch b of B=4):
    xt      = x[b].T                                  # (N=512, D=96)
    g1      = xt @ Wg1.T                              # (512, 256)
    g2      = xt @ Wg2.T                              # (512, 256)
    score   = sum_a Wa[a] * tanh(g1[n,a] + g2[m,a] + bg[a])    # (512, 512)
    att     = sigmoid(score + Wa_b + ba)
    out[b]  = att @ xt                                # (512, 96)

Sharding: core c handles batch b = c//2 and query-rows n in
[(c%2)*256, (c%2)*256+256).  Each core computes its full out rows; the
host concatenates.

Algorithm (v2, Fourier factorization): approximate
    tanh(u+v) ~= sum_{j=1..FJ} BJ[j-1] * sin(j*S*(u+v)),   S = pi/FL
(coefficients from a smoothness-regularized weighted least-squares fit
of tanh on |u+v|<=12 with free periodic completion).  Each harmonic
separates:  sin(jTu+jTv) = sin(jTu)cos(jTv) + cos(jTu)sin(jTv), so the
whole N x N score matrix becomes plain matmuls over a contraction dim
of (a, j, sin|cos) pairs:

  - theta = S*(g + bg) per side via PE matmuls (K=D=96).
  - base features sin(theta), cos(theta) via ACT Sin (args stay within
    the LUT's [-pi, pi] domain: |S*g| + pi/2 < pi for |g| <= FL/2).
  - harmonics via the Chebyshev recurrence f_j = 2cos(theta)*f_{j-1} -
    f_{j-2} on the Vector engine in fp16 (2 tensor_tensor ops per j
    over a combined [128, 2, 1536] tile holding both sides and both
    sin/cos lanes).
  - u-side features scaled by Wa[a]*BJ[j-1] (tensor_scalar, per-
    partition Wa vector + immediate).
  - scoring: per (j, fn, a-chunk, m-block) matmul with the v-side
    feature block as the stationary operand -> scoreT[m, n] accumulates
    into 4 PSUM banks [128, 256] fp32.
  - sigmoid (+Wa_b+ba) PSUM->SBUF fp16 yields attT[m, n] directly, the
    lhsT of the final out[n, d] matmul against x[b].T (fp16).
"""

import numpy as np

B, D, N, A = 4, 96, 512, 256
NH = N // 2          # query rows per core
NCORES = 8

FJ = 13
FL = 13.0
FS = float(np.pi / FL)
BJ = [1.25339337, -0.01945643, 0.37026378, -0.0301986, 0.17909742,
      -0.02951455, 0.0971231, -0.02324765, 0.05208288, -0.01869683,
      0.02430917, -0.01051296, 0.01973076]

_cache = {}


def _build_nc_v2():
    import concourse.bacc as bacc
    import concourse.mybir as mybir
    from concourse import tile

    f32 = mybir.dt.float32
    f16 = mybir.dt.float16
    AF = mybir.ActivationFunctionType
    MULT = mybir.AluOpType.mult

    nc = bacc.Bacc("TRN2", target_bir_lowering=False)

    # packed inputs (fp32: the fp16 variant shifts SBUF tile addresses
    # into a layout that slows DVE tensor_tensor ops by ~20%)
    import os
    _fin = f32 if int(os.environ.get("K_F32IN", "0")) else f16
    vin = nc.dram_tensor("vin", [D, A + N], _fin, kind="ExternalInput")
    uin = nc.dram_tensor("uin", [D, A + NH], _fin, kind="ExternalInput")
    biasv = nc.dram_tensor("biasv", [128, 7], f32, kind="ExternalInput")
    xkT = nc.dram_tensor("xkT", [N, D], f16, kind="ExternalInput")
    out = nc.dram_tensor("out", [NH, D], f32, kind="ExternalOutput")

    with tile.TileContext(nc) as tc:
        with (
            tc.tile_pool(name="consts", bufs=1) as consts,
            tc.tile_pool(name="ufeat", bufs=1) as ufeat,
            tc.tile_pool(name="uscal", bufs=1) as uscal,
            tc.tile_pool(name="tmpp", bufs=2) as tmpp,
            tc.tile_pool(name="gps", bufs=2, space="PSUM") as gps,
            tc.tile_pool(name="scps", bufs=1, space="PSUM") as scps,
            tc.tile_pool(name="fps", bufs=1, space="PSUM") as fps,
            tc.tile_pool(name="attp", bufs=1) as attp,
            tc.tile_pool(name="opool", bufs=1) as opool,
        ):
            vin_sb = consts.tile([D, A + N], _fin, tag="vin")
            uin_sb = consts.tile([D, A + NH], _fin, tag="uin")
            biasv_sb = consts.tile([128, 7], f32, tag="biasv")
            xkT_sb = consts.tile([128, 4, D], f16, tag="xkT")
            w2_sb = vin_sb[:, :A]
            xk_sb = vin_sb[:, A:A + N]
            w1_sb = uin_sb[:, :A]
            xq_sb = uin_sb[:, A:A + NH]
            bsin_sb = biasv_sb[:, 0:2]
            bcos_sb = biasv_sb[:, 2:4]
            wav_sb = biasv_sb[:, 4:6]
            sgb_sb = biasv_sb[:, 6:7]

            # dummy Sin on garbage to preload ACT table sets during DMAs
            dummy = consts.tile([128, 1], f32, tag="dummy")
            nc.gpsimd.memset(dummy[:], 0.0)
            nc.scalar.activation(dummy[:], dummy[:], AF.Sin)

            nc.sync.dma_start(vin_sb[:], vin.ap())
            nc.sync.dma_start(biasv_sb[:], biasv.ap())
            nc.sync.dma_start(uin_sb[:], uin.ap())
            nc.scalar.dma_start(
                xkT_sb[:], xkT.ap().rearrange("(mb p) d -> p mb d", p=128)
            )

            # combined feature tiles, j = 1..FJ:
            # [128, (sin|cos), v-part(c*512+m) | u-part(1024 + c*256+n)]
            FV = N * 2            # 1024: v-part width
            FT = FV + NH * 2      # 1536: total width
            cf = [ufeat.tile([128, 2, FT], f16, tag=f"cf{j}", name=f"cf{j}")
                  if j >= 1 else None for j in range(FJ + 1)]
            us = [uscal.tile([128, 2, NH * 2], f16, tag=f"us{j}", name=f"us{j}")
                  if j >= 1 else None for j in range(FJ + 1)]
            twoc = consts.tile([128, 2, FT], f16, tag="twoc")

            # theta tiles + base features (j=1); v-side first, cos first
            thvs = []
            for c in range(2):
                thv = gps.tile([128, N], f32, tag="th", name=f"thv{c}")
                nc.tensor.matmul(thv[:], w2_sb[:, c * 128:(c + 1) * 128],
                                 xk_sb[:])
                thvs.append(thv)
            thus = []
            for c in range(2):
                thu = gps.tile([128, N], f32, tag="th", name=f"thu{c}")
                nc.tensor.matmul(thu[:, :NH], w1_sb[:, c * 128:(c + 1) * 128],
                                 xq_sb[:])
                thus.append(thu)
            for c in range(2):
                nc.scalar.activation(cf[1][:, 1, c * N:(c + 1) * N],
                                     thvs[c][:], AF.Sin,
                                     bias=bcos_sb[:, c:c + 1])
                nc.scalar.activation(cf[1][:, 0, c * N:(c + 1) * N],
                                     thvs[c][:], AF.Sin,
                                     bias=bsin_sb[:, c:c + 1])
            for c in range(2):
                nc.scalar.activation(cf[1][:, 1, FV + c * NH:FV + (c + 1) * NH],
                                     thus[c][:, :NH], AF.Sin,
                                     bias=bcos_sb[:, c:c + 1])
                nc.scalar.activation(cf[1][:, 0, FV + c * NH:FV + (c + 1) * NH],
                                     thus[c][:, :NH], AF.Sin,
                                     bias=bsin_sb[:, c:c + 1])

            for fn in range(2):
                nc.vector.tensor_scalar_mul(twoc[:, fn, :], cf[1][:, 1, :], 2.0)

            sc = [scps.tile([128, NH], f32, tag=f"sc{mb}", name=f"sc{mb}")
                  for mb in range(4)]

            for j in range(1, FJ + 1):
                if j == 2:
                    # f_2 = 2c*f_1 - f_0 with f_0 = (0, 1)
                    tmpc = tmpp.tile([128, 2, FT], f16, tag="tmpc")
                    nc.vector.tensor_mul(tmpc[:], cf[1][:], twoc[:])
                    nc.vector.tensor_copy(cf[2][:, 0, :], tmpc[:, 0, :])
                    nc.vector.tensor_scalar_add(cf[2][:, 1, :], tmpc[:, 1, :],
                                                -1.0)
                elif j >= 3:
                    tmpc = tmpp.tile([128, 2, FT], f16, tag="tmpc")
                    nc.vector.tensor_mul(tmpc[:], cf[j - 1][:], twoc[:])
                    nc.vector.tensor_sub(cf[j][:], tmpc[:], cf[j - 2][:])
                # scale u-part by Wa[a]*BJ[j-1] (both fn halves per op)
                for c in range(2):
                    nc.vector.tensor_scalar(
                        us[j][:, :, c * NH:(c + 1) * NH],
                        cf[j][:, :, FV + c * NH:FV + (c + 1) * NH],
                        wav_sb[:, c:c + 1], float(BJ[j - 1]),
                        MULT, MULT,
                    )
                # scoring: sin_u pairs cos_v, cos_u pairs sin_v
                for fn in range(2):
                    for c in range(2):
                        for mb in range(4):
                            nc.tensor.matmul(
                                sc[mb][:],
                                cf[j][:, 1 - fn,
                                      c * N + mb * 128: c * N + (mb + 1) * 128],
                                us[j][:, fn, c * NH:(c + 1) * NH],
                                start=(j == 1 and fn == 0 and c == 0),
                                stop=(j == FJ and fn == 1 and c == 1),
                                skip_group_check=True,
                            )

            attT = attp.tile([128, 4, NH], f16, tag="attT")
            out_sb = opool.tile([128, 2, D], f32, tag="out")
            fos = [fps.tile([128, D], f32, tag=f"fo{nb}", name=f"fo{nb}")
                   for nb in range(2)]
            for mb in range(4):
                nc.scalar.activation(
                    attT[:, mb, :], sc[mb][:], AF.Sigmoid, bias=sgb_sb[:, 0:1]
                )
                for nb in range(2):
                    nc.tensor.matmul(
                        fos[nb][:],
                        attT[:, mb, nb * 128:(nb + 1) * 128],
                        xkT_sb[:, mb, :],
                        start=(mb == 0),
                        stop=(mb == 3),
                        skip_group_check=True,
                    )
            for nb in range(2):
                nc.vector.tensor_copy(out_sb[:, nb, :], fos[nb][:])

            nc.sync.dma_start(
                out.ap().rearrange("(nb p) d -> p nb d", p=128), out_sb[:]
            )

    nc.compile()
    return nc


def _prep_inputs_v2(x, Wg1, Wg2, bg, Wa_w, Wa_b, ba):
    """Host-side packing/slicing only (no reference math)."""
    x = np.asarray(x, np.float32)
    w1s = FS * np.asarray(Wg1, np.float32).T
    w2s = FS * np.asarray(Wg2, np.float32).T
    bgv = FS * np.asarray(bg, np.float32)
    biasv = np.empty((128, 7), np.float32)
    biasv[:, 0:2] = bgv.reshape(2, 128).T
    biasv[:, 2:4] = bgv.reshape(2, 128).T + np.float32(np.pi / 2)
    biasv[:, 4:6] = np.asarray(Wa_w, np.float32).reshape(2, 128).T
    biasv[:, 6] = float(np.asarray(Wa_b).ravel()[0]) \
        + float(np.asarray(ba).ravel()[0])
    in_maps = []
    for c in range(NCORES):
        b, half = c // 2, c % 2
        xb = x[b]
        import os
        dt = np.float32 if int(os.environ.get("K_F32IN", "0")) else np.float16
        vin = np.ascontiguousarray(np.concatenate([w2s, xb], axis=1), dtype=dt)
        uin = np.ascontiguousarray(
            np.concatenate([w1s, xb[:, half * NH:(half + 1) * NH]], axis=1),
            dtype=dt)
        in_maps.append({
            "vin": vin,
            "uin": uin,
            "biasv": np.ascontiguousarray(biasv),
            "xkT": np.ascontiguousarray(xb.T.astype(np.float16)),
        })
    return in_maps


def _run(inputs, trace=False):
    from concourse.bass_utils import run_bass_kernel_spmd

    if "nc" not in _cache:
        _cache["nc"] = _build_nc_v2()
    nc = _cache["nc"]
    in_maps = _prep_inputs_v2(**inputs)
    res = run_bass_kernel_spmd(
        nc, in_maps, core_ids=list(range(NCORES)), trace=trace
    )
    out = np.empty((B, N, D), np.float32)
    for c in range(NCORES):
        b, half = c // 2, c % 2
        out[b, half * NH:(half + 1) * NH] = res.results[c]["out"]
    return out, res


def kernel(**inputs):
    out, _ = _run(inputs, trace=False)
    return out


# revision 24
# speedup vs baseline: 1.1399x; 1.0642x over previous
"""Additive-attention kernel for Trainium2 (8 NeuronCores, SPMD).

Problem (per batch b of B=4):
    xt      = x[b].T                                  # (N=512, D=96)
    g1      = xt @ Wg1.T                              # (512, 256)
    g2      = xt @ Wg2.T                              # (512, 256)
    score   = sum_a Wa[a] * tanh(g1[n,a] + g2[m,a] + bg[a])    # (512, 512)
    att     = sigmoid(score + Wa_b + ba)
    out[b]  = att @ xt                                # (512, 96)

Sharding: core c handles batch b = c//2 and query-rows n in
[(c%2)*256, (c%2)*256+256).  Each core computes its full out rows; the
host concatenates.

Algorithm (v2, Fourier factorization): approximate
    tanh(u+v) ~= sum_{j=1..FJ} BJ[j-1] * sin(j*S*(u+v)),   S = pi/FL
(coefficients from a smoothness-regularized weighted least-squares fit
of tanh on |u+v|<=12 with free periodic completion).  Each harmonic
separates:  sin(jTu+jTv) = sin(jTu)cos(jTv) + cos(jTu)sin(jTv), so the
whole N x N score matrix becomes plain matmuls over a contraction dim
of (a, j, sin|cos) pairs:

  - theta = S*(g + bg) per side via PE matmuls (K=D=96).
  - base features sin(theta), cos(theta) via ACT Sin (args stay within
    the LUT's [-pi, pi] domain: |S*g| + pi/2 < pi for |g| <= FL/2).
  - harmonics via the Chebyshev recurrence f_j = 2cos(theta)*f_{j-1} -
    f_{j-2} on the Vector engine in fp16 (2 tensor_tensor ops per j
    over a combined [128, 2, 1536] tile holding both sides and both
    sin/cos lanes).
  - u-side features scaled by Wa[a]*BJ[j-1] (tensor_scalar, per-
    partition Wa vector + immediate).
  - scoring: per (j, fn, a-chunk, m-block) matmul with the v-side
    feature block as the stationary operand -> scoreT[m, n] accumulates
    into 4 PSUM banks [128, 256] fp32.
  - sigmoid (+Wa_b+ba) PSUM->SBUF fp16 yields attT[m, n] directly, the
    lhsT of the final out[n, d] matmul against x[b].T (fp16).
"""

import numpy as np

B, D, N, A = 4, 96, 512, 256
NH = N // 2          # query rows per core
NCORES = 8

FJ = 12
FL = 12.0
FS = float(np.pi / FL)
BJ = [1.25456309, -0.03455722, 0.37834737, -0.05286062, 0.19589199,
      -0.05558991, 0.10654432, -0.03719506, 0.04349812, -0.00692619,
      0.01704596, 0.00347006]

_cache = {}


def _build_nc_v2(bg_zero=False):
    import concourse.bacc as bacc
    import concourse.mybir as mybir
    from concourse import tile

    f32 = mybir.dt.float32
    f16 = mybir.dt.float16
    AF = mybir.ActivationFunctionType
    MULT = mybir.AluOpType.mult

    nc = bacc.Bacc("TRN2", target_bir_lowering=False)

    # packed inputs (fp32: the fp16 variant shifts SBUF tile addresses
    # into a layout that slows DVE tensor_tensor ops by ~20%)
    import os
    _fin = f32 if int(os.environ.get("K_F32IN", "0")) else f16
    vin = nc.dram_tensor("vin", [D, A + N], _fin, kind="ExternalInput")
    uin = nc.dram_tensor("uin", [D, A + NH], _fin, kind="ExternalInput")
    biasv = nc.dram_tensor("biasv", [128, 7], f32, kind="ExternalInput")
    xkT = nc.dram_tensor("xkT", [N, D], f16, kind="ExternalInput")
    out = nc.dram_tensor("out", [NH, D], f32, kind="ExternalOutput")

    with tile.TileContext(nc) as tc:
        with (
            tc.tile_pool(name="consts", bufs=1) as consts,
            tc.tile_pool(name="ufeat", bufs=1) as ufeat,
            tc.tile_pool(name="uscal", bufs=1) as uscal,
            tc.tile_pool(name="tmpp", bufs=2) as tmpp,
            tc.tile_pool(name="gps", bufs=2, space="PSUM") as gps,
            tc.tile_pool(name="scps", bufs=1, space="PSUM") as scps,
            tc.tile_pool(name="attp", bufs=1) as attp,
            tc.tile_pool(name="opool", bufs=1) as opool,
        ):
            vin_sb = consts.tile([D, A + N], _fin, tag="vin")
            uin_sb = consts.tile([D, A + NH], _fin, tag="uin")
            biasv_sb = consts.tile([128, 7], f32, tag="biasv")
            xkT_sb = consts.tile([128, 4, D], f16, tag="xkT")
            w2_sb = vin_sb[:, :A]
            xk_sb = vin_sb[:, A:A + N]
            w1_sb = uin_sb[:, :A]
            xq_sb = uin_sb[:, A:A + NH]
            bsin_sb = biasv_sb[:, 0:2]
            bcos_sb = biasv_sb[:, 2:4]
            wav_sb = biasv_sb[:, 4:6]
            sgb_sb = biasv_sb[:, 6:7]

            # dummy Sin on garbage to preload ACT table sets during DMAs
            dummy = consts.tile([128, 1], f32, tag="dummy")
            nc.gpsimd.memset(dummy[:], 0.0)
            nc.scalar.activation(dummy[:], dummy[:], AF.Sin)

            nc.sync.dma_start(vin_sb[:], vin.ap())
            nc.sync.dma_start(biasv_sb[:], biasv.ap())
            nc.sync.dma_start(uin_sb[:], uin.ap())
            nc.scalar.dma_start(
                xkT_sb[:], xkT.ap().rearrange("(mb p) d -> p mb d", p=128)
            )

            # combined feature tiles, j = 1..FJ:
            # [128, (sin|cos), v-part(c*512+m) | u-part(1024 + c*256+n)]
            FV = N * 2            # 1024: v-part width
            FT = FV + NH * 2      # 1536: total width
            cf = [ufeat.tile([128, 2, FT], f16, tag=f"cf{j}", name=f"cf{j}")
                  if j >= 1 else None for j in range(FJ + 1)]
            us = [uscal.tile([128, 2, NH * 2], f16, tag=f"us{j}", name=f"us{j}")
                  if j >= 1 else None for j in range(FJ + 1)]
            twoc = consts.tile([128, 2, FT], f16, tag="twoc")

            # theta tiles + base features (j=1); v-side first, cos first
            HPI = float(np.pi / 2)
            if bg_zero:
                # bg == 0: immediate biases, chunk-merged theta tiles/sins
                thv = gps.tile([128, FV], f32, tag="thv", name="thv", bufs=1)
                for c in range(2):
                    nc.tensor.matmul(thv[:, c * N:(c + 1) * N],
                                     w2_sb[:, c * 128:(c + 1) * 128], xk_sb[:])
                thu = gps.tile([128, N], f32, tag="thu", name="thu", bufs=1)
                for c in range(2):
                    nc.tensor.matmul(thu[:, c * NH:(c + 1) * NH],
                                     w1_sb[:, c * 128:(c + 1) * 128], xq_sb[:])
                nc.scalar.activation(cf[1][:, 1, :FV], thv[:], AF.Sin,
                                     bias=bcos_sb[:, 0:1])
                nc.scalar.activation(cf[1][:, 0, :FV], thv[:], AF.Sin,
                                     bias=bsin_sb[:, 0:1])
                nc.scalar.activation(cf[1][:, 1, FV:], thu[:], AF.Sin,
                                     bias=bcos_sb[:, 0:1])
                nc.scalar.activation(cf[1][:, 0, FV:], thu[:], AF.Sin,
                                     bias=bsin_sb[:, 0:1])
            else:
                thvs = []
                for c in range(2):
                    thv = gps.tile([128, N], f32, tag="th", name=f"thv{c}")
                    nc.tensor.matmul(thv[:], w2_sb[:, c * 128:(c + 1) * 128],
                                     xk_sb[:])
                    thvs.append(thv)
                thus = []
                for c in range(2):
                    thu = gps.tile([128, N], f32, tag="th", name=f"thu{c}")
                    nc.tensor.matmul(thu[:, :NH],
                                     w1_sb[:, c * 128:(c + 1) * 128], xq_sb[:])
                    thus.append(thu)
                for c in range(2):
                    nc.scalar.activation(cf[1][:, 1, c * N:(c + 1) * N],
                                         thvs[c][:], AF.Sin,
                                         bias=bcos_sb[:, c:c + 1])
                    nc.scalar.activation(cf[1][:, 0, c * N:(c + 1) * N],
                                         thvs[c][:], AF.Sin,
                                         bias=bsin_sb[:, c:c + 1])
                for c in range(2):
                    nc.scalar.activation(
                        cf[1][:, 1, FV + c * NH:FV + (c + 1) * NH],
                        thus[c][:, :NH], AF.Sin, bias=bcos_sb[:, c:c + 1])
                    nc.scalar.activation(
                        cf[1][:, 0, FV + c * NH:FV + (c + 1) * NH],
                        thus[c][:, :NH], AF.Sin, bias=bsin_sb[:, c:c + 1])

            for fn in range(2):
                nc.vector.tensor_scalar_mul(twoc[:, fn, :], cf[1][:, 1, :], 2.0)

            sc = [scps.tile([128, NH], f32, tag=f"sc{mb}", name=f"sc{mb}")
                  for mb in range(4)]

            for j in range(1, FJ + 1):
                if j == 2:
                    # f_2 = 2c*f_1 - f_0 with f_0 = (0, 1)
                    tmpc = tmpp.tile([128, 2, FT], f16, tag="tmpc")
                    nc.vector.tensor_mul(tmpc[:], cf[1][:], twoc[:])
                    nc.vector.tensor_copy(cf[2][:, 0, :], tmpc[:, 0, :])
                    nc.vector.tensor_scalar_add(cf[2][:, 1, :], tmpc[:, 1, :],
                                                -1.0)
                elif j >= 3:
                    tmpc = tmpp.tile([128, 2, FT], f16, tag="tmpc")
                    nc.vector.tensor_mul(tmpc[:], cf[j - 1][:], twoc[:])
                    nc.vector.tensor_sub(cf[j][:], tmpc[:], cf[j - 2][:])
                # scale u-part by Wa[a]*BJ[j-1] (both fn halves per op)
                for c in range(2):
                    nc.vector.tensor_scalar(
                        us[j][:, :, c * NH:(c + 1) * NH],
                        cf[j][:, :, FV + c * NH:FV + (c + 1) * NH],
                        wav_sb[:, c:c + 1], float(BJ[j - 1]),
                        MULT, MULT,
                    )
                # scoring: sin_u pairs cos_v, cos_u pairs sin_v
                for fn in range(2):
                    for c in range(2):
                        for mb in range(4):
                            nc.tensor.matmul(
                                sc[mb][:],
                                cf[j][:, 1 - fn,
                                      c * N + mb * 128: c * N + (mb + 1) * 128],
                                us[j][:, fn, c * NH:(c + 1) * NH],
                                start=(j == 1 and fn == 0 and c == 0),
                                stop=(j == FJ and fn == 1 and c == 1),
                                skip_group_check=True,
                            )

            attT = attp.tile([128, 4, NH], f16, tag="attT")
            out_sb = opool.tile([128, 2, D], f32, tag="out")
            # reuse the (dead) theta-tile PSUM slots for the final accums
            if bg_zero:
                fos = [gps.tile([128, D], f32, tag="thv", name="fo0", bufs=1),
                       gps.tile([128, D], f32, tag="thu", name="fo1", bufs=1)]
            else:
                fos = [gps.tile([128, D], f32, tag="th", name=f"fo{nb}")
                       for nb in range(2)]
            for mb in range(4):
                nc.scalar.activation(
                    attT[:, mb, :], sc[mb][:], AF.Sigmoid, bias=sgb_sb[:, 0:1]
                )
                for nb in range(2):
                    nc.tensor.matmul(
                        fos[nb][:],
                        attT[:, mb, nb * 128:(nb + 1) * 128],
                        xkT_sb[:, mb, :],
                        start=(mb == 0),
                        stop=(mb == 3),
                        skip_group_check=True,
                    )
            for nb in range(2):
                nc.vector.tensor_copy(out_sb[:, nb, :], fos[nb][:])

            nc.sync.dma_start(
                out.ap().rearrange("(nb p) d -> p nb d", p=128), out_sb[:]
            )

    nc.compile()
    return nc


def _prep_inputs_v2(x, Wg1, Wg2, bg, Wa_w, Wa_b, ba):
    """Host-side packing/slicing only (no reference math)."""
    x = np.asarray(x, np.float32)
    w1s = FS * np.asarray(Wg1, np.float32).T
    w2s = FS * np.asarray(Wg2, np.float32).T
    bgv = FS * np.asarray(bg, np.float32)
    biasv = np.empty((128, 7), np.float32)
    biasv[:, 0:2] = bgv.reshape(2, 128).T
    biasv[:, 2:4] = bgv.reshape(2, 128).T + np.float32(np.pi / 2)
    biasv[:, 4:6] = np.asarray(Wa_w, np.float32).reshape(2, 128).T
    biasv[:, 6] = float(np.asarray(Wa_b).ravel()[0]) \
        + float(np.asarray(ba).ravel()[0])
    in_maps = []
    for c in range(NCORES):
        b, half = c // 2, c % 2
        xb = x[b]
        import os
        dt = np.float32 if int(os.environ.get("K_F32IN", "0")) else np.float16
        vin = np.ascontiguousarray(np.concatenate([w2s, xb], axis=1), dtype=dt)
        uin = np.ascontiguousarray(
            np.concatenate([w1s, xb[:, half * NH:(half + 1) * NH]], axis=1),
            dtype=dt)
        in_maps.append({
            "vin": vin,
            "uin": uin,
            "biasv": np.ascontiguousarray(biasv),
            "xkT": np.ascontiguousarray(xb.T.astype(np.float16)),
        })
    return in_maps


def _run(inputs, trace=False):
    from concourse.bass_utils import run_bass_kernel_spmd

    bg_zero = bool(np.all(np.asarray(inputs["bg"]) == 0))
    key = ("nc", bg_zero)
    if key not in _cache:
        _cache[key] = _build_nc_v2(bg_zero=bg_zero)
    nc = _cache[key]
    in_maps = _prep_inputs_v2(**inputs)
    res = run_bass_kernel_spmd(
        nc, in_maps, core_ids=list(range(NCORES)), trace=trace
    )
    out = np.empty((B, N, D), np.float32)
    for c in range(NCORES):
        b, half = c // 2, c % 2
        out[b, half * NH:(half + 1) * NH] = res.results[c]["out"]
    return out, res


def kernel(**inputs):
    out, _ = _run(inputs, trace=False)
    return out


# revision 25
# speedup vs baseline: 1.2123x; 1.0635x over previous
"""Additive-attention kernel for Trainium2 (8 NeuronCores, SPMD).

Problem (per batch b of B=4):
    xt      = x[b].T                                  # (N=512, D=96)
    g1      = xt @ Wg1.T                              # (512, 256)
    g2      = xt @ Wg2.T                              # (512, 256)
    score   = sum_a Wa[a] * tanh(g1[n,a] + g2[m,a] + bg[a])    # (512, 512)
    att     = sigmoid(score + Wa_b + ba)
    out[b]  = att @ xt                                # (512, 96)

Sharding: core c handles batch b = c//2 and query-rows n in
[(c%2)*256, (c%2)*256+256).  Each core computes its full out rows; the
host concatenates.

Algorithm (v2, Fourier factorization): approximate
    tanh(u+v) ~= sum_{j=1..FJ} BJ[j-1] * sin(j*S*(u+v)),   S = pi/FL
(coefficients from a smoothness-regularized weighted least-squares fit
of tanh on |u+v|<=12 with free periodic completion).  Each harmonic
separates:  sin(jTu+jTv) = sin(jTu)cos(jTv) + cos(jTu)sin(jTv), so the
whole N x N score matrix becomes plain matmuls over a contraction dim
of (a, j, sin|cos) pairs:

  - theta = S*(g + bg) per side via PE matmuls (K=D=96).
  - base features sin(theta), cos(theta) via ACT Sin (args stay within
    the LUT's [-pi, pi] domain: |S*g| + pi/2 < pi for |g| <= FL/2).
  - harmonics via the Chebyshev recurrence f_j = 2cos(theta)*f_{j-1} -
    f_{j-2} on the Vector engine in fp16 (2 tensor_tensor ops per j
    over a combined [128, 2, 1536] tile holding both sides and both
    sin/cos lanes).
  - u-side features scaled by Wa[a]*BJ[j-1] (tensor_scalar, per-
    partition Wa vector + immediate).
  - scoring: per (j, fn, a-chunk, m-block) matmul with the v-side
    feature block as the stationary operand -> scoreT[m, n] accumulates
    into 4 PSUM banks [128, 256] fp32.
  - sigmoid (+Wa_b+ba) PSUM->SBUF fp16 yields attT[m, n] directly, the
    lhsT of the final out[n, d] matmul against x[b].T (fp16).
"""

import numpy as np

B, D, N, A = 4, 96, 512, 256
NH = N // 2          # query rows per core
NCORES = 8

FJ = 12
FL = 12.0
FS = float(np.pi / FL)
BJ = [1.25456309, -0.03455722, 0.37834737, -0.05286062, 0.19589199,
      -0.05558991, 0.10654432, -0.03719506, 0.04349812, -0.00692619,
      0.01704596, 0.00347006]

_cache = {}


def _build_nc_v2(bg_zero=False):
    import concourse.bacc as bacc
    import concourse.mybir as mybir
    from concourse import tile

    f32 = mybir.dt.float32
    f16 = mybir.dt.float16
    AF = mybir.ActivationFunctionType
    MULT = mybir.AluOpType.mult

    nc = bacc.Bacc("TRN2", target_bir_lowering=False)

    # packed inputs (fp32: the fp16 variant shifts SBUF tile addresses
    # into a layout that slows DVE tensor_tensor ops by ~20%)
    import os
    _fin = f32 if int(os.environ.get("K_F32IN", "0")) else f16
    vin = nc.dram_tensor("vin", [D, A + N], _fin, kind="ExternalInput")
    uin = nc.dram_tensor("uin", [D, A + NH], _fin, kind="ExternalInput")
    biasv = nc.dram_tensor("biasv", [128, 7 + 2 * FJ], f32, kind="ExternalInput")
    xkT = nc.dram_tensor("xkT", [N, D], f16, kind="ExternalInput")
    out = nc.dram_tensor("out", [NH, D], f32, kind="ExternalOutput")

    with tile.TileContext(nc) as tc:
        with (
            tc.tile_pool(name="consts", bufs=1) as consts,
            tc.tile_pool(name="ufeat", bufs=1) as ufeat,
            tc.tile_pool(name="uscal", bufs=1) as uscal,
            tc.tile_pool(name="tmpp", bufs=2) as tmpp,
            tc.tile_pool(name="gps", bufs=2, space="PSUM") as gps,
            tc.tile_pool(name="scps", bufs=1, space="PSUM") as scps,
            tc.tile_pool(name="attp", bufs=1) as attp,
            tc.tile_pool(name="opool", bufs=1) as opool,
        ):
            vin_sb = consts.tile([D, A + N], _fin, tag="vin")
            uin_sb = consts.tile([D, A + NH], _fin, tag="uin")
            biasv_sb = consts.tile([128, 7 + 2 * FJ], f32, tag="biasv")
            xkT_sb = consts.tile([128, 4, D], f16, tag="xkT")
            w2_sb = vin_sb[:, :A]
            xk_sb = vin_sb[:, A:A + N]
            w1_sb = uin_sb[:, :A]
            xq_sb = uin_sb[:, A:A + NH]
            bsin_sb = biasv_sb[:, 0:2]
            bcos_sb = biasv_sb[:, 2:4]
            wav_sb = biasv_sb[:, 4:6]
            sgb_sb = biasv_sb[:, 6:7]

            # dummy Sin on garbage to preload ACT table sets during DMAs
            dummy = consts.tile([128, 1], f32, tag="dummy")
            nc.gpsimd.memset(dummy[:], 0.0)
            nc.scalar.activation(dummy[:], dummy[:], AF.Sin)

            nc.sync.dma_start(vin_sb[:], vin.ap())
            nc.sync.dma_start(biasv_sb[:], biasv.ap())
            nc.sync.dma_start(uin_sb[:], uin.ap())
            nc.scalar.dma_start(
                xkT_sb[:], xkT.ap().rearrange("(mb p) d -> p mb d", p=128)
            )

            # combined feature tiles, j = 1..FJ:
            # [128, (sin|cos), v-part(c*512+m) | u-part(1024 + c*256+n)]
            FV = N * 2            # 1024: v-part width
            FT = FV + NH * 2      # 1536: total width
            cf = [ufeat.tile([128, 2, FT], f16, tag=f"cf{j}", name=f"cf{j}")
                  if j >= 1 else None for j in range(FJ + 1)]
            us = [uscal.tile([128, 2, NH * 2], f16, tag=f"us{j}", name=f"us{j}")
                  if j >= 1 else None for j in range(FJ + 1)]
            twoc = consts.tile([128, 2, FT], f16, tag="twoc")

            # theta tiles + base features (j=1); v-side first, cos first
            HPI = float(np.pi / 2)
            if bg_zero:
                # bg == 0: immediate biases, chunk-merged theta tiles/sins
                thv = gps.tile([128, FV], f32, tag="thv", name="thv", bufs=1)
                for c in range(2):
                    nc.tensor.matmul(thv[:, c * N:(c + 1) * N],
                                     w2_sb[:, c * 128:(c + 1) * 128], xk_sb[:])
                thu = gps.tile([128, N], f32, tag="thu", name="thu", bufs=1)
                for c in range(2):
                    nc.tensor.matmul(thu[:, c * NH:(c + 1) * NH],
                                     w1_sb[:, c * 128:(c + 1) * 128], xq_sb[:])
                nc.scalar.activation(cf[1][:, 1, :FV], thv[:], AF.Sin,
                                     bias=bcos_sb[:, 0:1])
                nc.scalar.activation(cf[1][:, 0, :FV], thv[:], AF.Sin,
                                     bias=bsin_sb[:, 0:1])
                nc.scalar.activation(cf[1][:, 1, FV:], thu[:], AF.Sin,
                                     bias=bcos_sb[:, 0:1])
                nc.scalar.activation(cf[1][:, 0, FV:], thu[:], AF.Sin,
                                     bias=bsin_sb[:, 0:1])
            else:
                thvs = []
                for c in range(2):
                    thv = gps.tile([128, N], f32, tag="th", name=f"thv{c}")
                    nc.tensor.matmul(thv[:], w2_sb[:, c * 128:(c + 1) * 128],
                                     xk_sb[:])
                    thvs.append(thv)
                thus = []
                for c in range(2):
                    thu = gps.tile([128, N], f32, tag="th", name=f"thu{c}")
                    nc.tensor.matmul(thu[:, :NH],
                                     w1_sb[:, c * 128:(c + 1) * 128], xq_sb[:])
                    thus.append(thu)
                for c in range(2):
                    nc.scalar.activation(cf[1][:, 1, c * N:(c + 1) * N],
                                         thvs[c][:], AF.Sin,
                                         bias=bcos_sb[:, c:c + 1])
                    nc.scalar.activation(cf[1][:, 0, c * N:(c + 1) * N],
                                         thvs[c][:], AF.Sin,
                                         bias=bsin_sb[:, c:c + 1])
                for c in range(2):
                    nc.scalar.activation(
                        cf[1][:, 1, FV + c * NH:FV + (c + 1) * NH],
                        thus[c][:, :NH], AF.Sin, bias=bcos_sb[:, c:c + 1])
                    nc.scalar.activation(
                        cf[1][:, 0, FV + c * NH:FV + (c + 1) * NH],
                        thus[c][:, :NH], AF.Sin, bias=bsin_sb[:, c:c + 1])

            for fn in range(2):
                nc.vector.tensor_scalar_mul(twoc[:, fn, :], cf[1][:, 1, :], 2.0)

            sc = [scps.tile([128, NH], f32, tag=f"sc{mb}", name=f"sc{mb}")
                  for mb in range(4)]

            for j in range(1, FJ + 1):
                if j == 2:
                    # f_2 = 2c*f_1 - f_0 with f_0 = (0, 1)
                    tmpc = tmpp.tile([128, 2, FT], f16, tag="tmpc")
                    nc.vector.tensor_mul(tmpc[:], cf[1][:], twoc[:])
                    nc.vector.tensor_copy(cf[2][:, 0, :], tmpc[:, 0, :])
                    nc.vector.tensor_scalar_add(cf[2][:, 1, :], tmpc[:, 1, :],
                                                -1.0)
                elif j >= 3:
                    tmpc = tmpp.tile([128, 2, FT], f16, tag="tmpc")
                    nc.vector.tensor_mul(tmpc[:], cf[j - 1][:], twoc[:])
                    nc.vector.tensor_sub(cf[j][:], tmpc[:], cf[j - 2][:])
                # scale u-part by Wa[a]*BJ[j-1] on the Scalar engine
                # (ACT Identity with per-partition scale; keeps DVE free)
                for c in range(2):
                    nc.scalar.activation(
                        us[j][:, :, c * NH:(c + 1) * NH],
                        cf[j][:, :, FV + c * NH:FV + (c + 1) * NH],
                        AF.Identity,
                        scale=biasv_sb[:, 7 + 2 * (j - 1) + c:
                                       8 + 2 * (j - 1) + c],
                    )
                # scoring: sin_u pairs cos_v, cos_u pairs sin_v
                for fn in range(2):
                    for c in range(2):
                        for mb in range(4):
                            nc.tensor.matmul(
                                sc[mb][:],
                                cf[j][:, 1 - fn,
                                      c * N + mb * 128: c * N + (mb + 1) * 128],
                                us[j][:, fn, c * NH:(c + 1) * NH],
                                start=(j == 1 and fn == 0 and c == 0),
                                stop=(j == FJ and fn == 1 and c == 1),
                                skip_group_check=True,
                            )

            attT = attp.tile([128, 4, NH], f16, tag="attT")
            out_sb = opool.tile([128, 2, D], f32, tag="out")
            # reuse the (dead) theta-tile PSUM slots for the final accums
            if bg_zero:
                fos = [gps.tile([128, D], f32, tag="thv", name="fo0", bufs=1),
                       gps.tile([128, D], f32, tag="thu", name="fo1", bufs=1)]
            else:
                fos = [gps.tile([128, D], f32, tag="th", name=f"fo{nb}")
                       for nb in range(2)]
            for mb in range(4):
                nc.scalar.activation(
                    attT[:, mb, :], sc[mb][:], AF.Sigmoid, bias=sgb_sb[:, 0:1]
                )
                for nb in range(2):
                    nc.tensor.matmul(
                        fos[nb][:],
                        attT[:, mb, nb * 128:(nb + 1) * 128],
                        xkT_sb[:, mb, :],
                        start=(mb == 0),
                        stop=(mb == 3),
                        skip_group_check=True,
                    )
            for nb in range(2):
                nc.vector.tensor_copy(out_sb[:, nb, :], fos[nb][:])

            nc.sync.dma_start(
                out.ap().rearrange("(nb p) d -> p nb d", p=128), out_sb[:]
            )

    nc.compile()
    return nc


def _prep_inputs_v2(x, Wg1, Wg2, bg, Wa_w, Wa_b, ba):
    """Host-side packing/slicing only (no reference math)."""
    x = np.asarray(x, np.float32)
    w1s = FS * np.asarray(Wg1, np.float32).T
    w2s = FS * np.asarray(Wg2, np.float32).T
    bgv = FS * np.asarray(bg, np.float32)
    biasv = np.empty((128, 7 + 2 * FJ), np.float32)
    biasv[:, 0:2] = bgv.reshape(2, 128).T
    biasv[:, 2:4] = bgv.reshape(2, 128).T + np.float32(np.pi / 2)
    biasv[:, 4:6] = np.asarray(Wa_w, np.float32).reshape(2, 128).T
    biasv[:, 6] = float(np.asarray(Wa_b).ravel()[0]) \
        + float(np.asarray(ba).ravel()[0])
    wac = np.asarray(Wa_w, np.float32).reshape(2, 128).T
    for j in range(1, FJ + 1):
        for c in range(2):
            biasv[:, 7 + 2 * (j - 1) + c] = wac[:, c] * np.float32(BJ[j - 1])
    in_maps = []
    for c in range(NCORES):
        b, half = c // 2, c % 2
        xb = x[b]
        import os
        dt = np.float32 if int(os.environ.get("K_F32IN", "0")) else np.float16
        vin = np.ascontiguousarray(np.concatenate([w2s, xb], axis=1), dtype=dt)
        uin = np.ascontiguousarray(
            np.concatenate([w1s, xb[:, half * NH:(half + 1) * NH]], axis=1),
            dtype=dt)
        in_maps.append({
            "vin": vin,
            "uin": uin,
            "biasv": np.ascontiguousarray(biasv),
            "xkT": np.ascontiguousarray(xb.T.astype(np.float16)),
        })
    return in_maps


def _run(inputs, trace=False):
    from concourse.bass_utils import run_bass_kernel_spmd

    bg_zero = bool(np.all(np.asarray(inputs["bg"]) == 0))
    key = ("nc", bg_zero)
    if key not in _cache:
        _cache[key] = _build_nc_v2(bg_zero=bg_zero)
    nc = _cache[key]
    in_maps = _prep_inputs_v2(**inputs)
    res = run_bass_kernel_spmd(
        nc, in_maps, core_ids=list(range(NCORES)), trace=trace
    )
    out = np.empty((B, N, D), np.float32)
    for c in range(NCORES):
        b, half = c // 2, c % 2
        out[b, half * NH:(half + 1) * NH] = res.results[c]["out"]
    return out, res


def kernel(**inputs):
    out, _ = _run(inputs, trace=False)
    return out


# revision 26
# speedup vs baseline: 1.2582x; 1.0378x over previous
"""Additive-attention kernel for Trainium2 (8 NeuronCores, SPMD).

Problem (per batch b of B=4):
    xt      = x[b].T                                  # (N=512, D=96)
    g1      = xt @ Wg1.T                              # (512, 256)
    g2      = xt @ Wg2.T                              # (512, 256)
    score   = sum_a Wa[a] * tanh(g1[n,a] + g2[m,a] + bg[a])    # (512, 512)
    att     = sigmoid(score + Wa_b + ba)
    out[b]  = att @ xt                                # (512, 96)

Sharding: core c handles batch b = c//2 and query-rows n in
[(c%2)*256, (c%2)*256+256).  Each core computes its full out rows; the
host concatenates.

Algorithm (v2, Fourier factorization): approximate
    tanh(u+v) ~= sum_{j=1..FJ} BJ[j-1] * sin(j*S*(u+v)),   S = pi/FL
(coefficients from a smoothness-regularized weighted least-squares fit
of tanh on |u+v|<=12 with free periodic completion).  Each harmonic
separates:  sin(jTu+jTv) = sin(jTu)cos(jTv) + cos(jTu)sin(jTv), so the
whole N x N score matrix becomes plain matmuls over a contraction dim
of (a, j, sin|cos) pairs:

  - theta = S*(g + bg) per side via PE matmuls (K=D=96).
  - base features sin(theta), cos(theta) via ACT Sin (args stay within
    the LUT's [-pi, pi] domain: |S*g| + pi/2 < pi for |g| <= FL/2).
  - harmonics via the Chebyshev recurrence f_j = 2cos(theta)*f_{j-1} -
    f_{j-2} on the Vector engine in fp16 (2 tensor_tensor ops per j
    over a combined [128, 2, 1536] tile holding both sides and both
    sin/cos lanes).
  - u-side features scaled by Wa[a]*BJ[j-1] (tensor_scalar, per-
    partition Wa vector + immediate).
  - scoring: per (j, fn, a-chunk, m-block) matmul with the v-side
    feature block as the stationary operand -> scoreT[m, n] accumulates
    into 4 PSUM banks [128, 256] fp32.
  - sigmoid (+Wa_b+ba) PSUM->SBUF fp16 yields attT[m, n] directly, the
    lhsT of the final out[n, d] matmul against x[b].T (fp16).
"""

import numpy as np

B, D, N, A = 4, 96, 512, 256
NH = N // 2          # query rows per core
NCORES = 8

FJ = 12
FL = 12.0
FS = float(np.pi / FL)
BJ = [1.25456309, -0.03455722, 0.37834737, -0.05286062, 0.19589199,
      -0.05558991, 0.10654432, -0.03719506, 0.04349812, -0.00692619,
      0.01704596, 0.00347006]

_cache = {}


def _build_nc_v2(bg_zero=False):
    import concourse.bacc as bacc
    import concourse.mybir as mybir
    from concourse import tile

    f32 = mybir.dt.float32
    f16 = mybir.dt.float16
    AF = mybir.ActivationFunctionType
    MULT = mybir.AluOpType.mult

    nc = bacc.Bacc("TRN2", target_bir_lowering=False)

    # packed inputs (fp32: the fp16 variant shifts SBUF tile addresses
    # into a layout that slows DVE tensor_tensor ops by ~20%)
    import os
    _fin = f32 if int(os.environ.get("K_F32IN", "0")) else f16
    vin = nc.dram_tensor("vin", [D, A + N], _fin, kind="ExternalInput")
    uin = nc.dram_tensor("uin", [D, A + NH], _fin, kind="ExternalInput")
    biasv = nc.dram_tensor("biasv", [128, 7 + 2 * FJ], f32, kind="ExternalInput")
    xkT = nc.dram_tensor("xkT", [N, D], f16, kind="ExternalInput")
    out = nc.dram_tensor("out", [NH, D], f32, kind="ExternalOutput")

    with tile.TileContext(nc) as tc:
        with (
            tc.tile_pool(name="consts", bufs=1) as consts,
            tc.tile_pool(name="ufeat", bufs=1) as ufeat,
            tc.tile_pool(name="uscal", bufs=1) as uscal,
            tc.tile_pool(name="tmpp", bufs=2) as tmpp,
            tc.tile_pool(name="gps", bufs=2, space="PSUM") as gps,
            tc.tile_pool(name="scps", bufs=1, space="PSUM") as scps,
            tc.tile_pool(name="attp", bufs=1) as attp,
            tc.tile_pool(name="opool", bufs=1) as opool,
        ):
            vin_sb = consts.tile([D, A + N], _fin, tag="vin")
            uin_sb = consts.tile([D, A + NH], _fin, tag="uin")
            biasv_sb = consts.tile([128, 7 + 2 * FJ], f32, tag="biasv")
            xkT_sb = consts.tile([128, 4, D], f16, tag="xkT")
            w2_sb = vin_sb[:, :A]
            xk_sb = vin_sb[:, A:A + N]
            w1_sb = uin_sb[:, :A]
            xq_sb = uin_sb[:, A:A + NH]
            bsin_sb = biasv_sb[:, 0:2]
            bcos_sb = biasv_sb[:, 2:4]
            wav_sb = biasv_sb[:, 4:6]
            sgb_sb = biasv_sb[:, 6:7]

            # dummy Sin on garbage to preload ACT table sets during DMAs
            dummy = consts.tile([128, 1], f32, tag="dummy")
            nc.gpsimd.memset(dummy[:], 0.0)
            nc.scalar.activation(dummy[:], dummy[:], AF.Sin)

            nc.sync.dma_start(vin_sb[:], vin.ap())
            nc.sync.dma_start(biasv_sb[:], biasv.ap())
            nc.sync.dma_start(uin_sb[:], uin.ap())
            nc.scalar.dma_start(
                xkT_sb[:], xkT.ap().rearrange("(mb p) d -> p mb d", p=128)
            )

            # combined feature tiles, j = 1..FJ:
            # [128, (sin|cos), v-part(c*512+m) | u-part(1024 + c*256+n)]
            FV = N * 2            # 1024: v-part width
            FT = FV + NH * 2      # 1536: total width
            cf = [ufeat.tile([128, 2, FT], f16, tag=f"cf{j}", name=f"cf{j}")
                  if j >= 1 else None for j in range(FJ + 1)]
            us = [uscal.tile([128, 2, NH * 2], f16, tag=f"us{j}", name=f"us{j}")
                  if j >= 1 else None for j in range(FJ + 1)]
            twoc = consts.tile([128, 2, FT], f16, tag="twoc")

            # theta tiles + base features (j=1); v-side first, cos first
            HPI = float(np.pi / 2)
            if bg_zero:
                # bg == 0: immediate biases, chunk-merged theta tiles/sins
                thv = gps.tile([128, FV], f32, tag="thv", name="thv", bufs=1)
                for c in range(2):
                    nc.tensor.matmul(thv[:, c * N:(c + 1) * N],
                                     w2_sb[:, c * 128:(c + 1) * 128], xk_sb[:])
                thu = gps.tile([128, N], f32, tag="thu", name="thu", bufs=1)
                for c in range(2):
                    nc.tensor.matmul(thu[:, c * NH:(c + 1) * NH],
                                     w1_sb[:, c * 128:(c + 1) * 128], xq_sb[:])
                nc.scalar.activation(cf[1][:, 1, :FV], thv[:], AF.Sin,
                                     bias=bcos_sb[:, 0:1])
                nc.scalar.activation(cf[1][:, 0, :FV], thv[:], AF.Sin,
                                     bias=bsin_sb[:, 0:1])
                nc.scalar.activation(cf[1][:, 1, FV:], thu[:], AF.Sin,
                                     bias=bcos_sb[:, 0:1])
                nc.scalar.activation(cf[1][:, 0, FV:], thu[:], AF.Sin,
                                     bias=bsin_sb[:, 0:1])
            else:
                thvs = []
                for c in range(2):
                    thv = gps.tile([128, N], f32, tag="th", name=f"thv{c}")
                    nc.tensor.matmul(thv[:], w2_sb[:, c * 128:(c + 1) * 128],
                                     xk_sb[:])
                    thvs.append(thv)
                thus = []
                for c in range(2):
                    thu = gps.tile([128, N], f32, tag="th", name=f"thu{c}")
                    nc.tensor.matmul(thu[:, :NH],
                                     w1_sb[:, c * 128:(c + 1) * 128], xq_sb[:])
                    thus.append(thu)
                for c in range(2):
                    nc.scalar.activation(cf[1][:, 1, c * N:(c + 1) * N],
                                         thvs[c][:], AF.Sin,
                                         bias=bcos_sb[:, c:c + 1])
                    nc.scalar.activation(cf[1][:, 0, c * N:(c + 1) * N],
                                         thvs[c][:], AF.Sin,
                                         bias=bsin_sb[:, c:c + 1])
                for c in range(2):
                    nc.scalar.activation(
                        cf[1][:, 1, FV + c * NH:FV + (c + 1) * NH],
                        thus[c][:, :NH], AF.Sin, bias=bcos_sb[:, c:c + 1])
                    nc.scalar.activation(
                        cf[1][:, 0, FV + c * NH:FV + (c + 1) * NH],
                        thus[c][:, :NH], AF.Sin, bias=bsin_sb[:, c:c + 1])

            for fn in range(2):
                nc.vector.tensor_scalar_mul(twoc[:, fn, :], cf[1][:, 1, :], 2.0)

            sc = [scps.tile([128, NH], f32, tag=f"sc{mb}", name=f"sc{mb}")
                  for mb in range(4)]

            for j in range(1, FJ + 1):
                if j == 2:
                    # f_2 = 2c*f_1 - f_0 with f_0 = (0, 1)
                    tmpc = tmpp.tile([128, 2, FT], f16, tag="tmpc")
                    nc.vector.tensor_mul(tmpc[:], cf[1][:], twoc[:])
                    nc.vector.tensor_copy(cf[2][:, 0, :], tmpc[:, 0, :])
                    nc.vector.tensor_scalar_add(cf[2][:, 1, :], tmpc[:, 1, :],
                                                -1.0)
                elif j == FJ:
                    # last harmonic: u-part first so the tail can start
                    tmpc = tmpp.tile([128, 2, FT], f16, tag="tmpc")
                    nc.vector.tensor_mul(tmpc[:], cf[j - 1][:], twoc[:])
                    nc.vector.tensor_sub(cf[j][:, :, FV:], tmpc[:, :, FV:],
                                         cf[j - 2][:, :, FV:])
                    nc.vector.tensor_sub(cf[j][:, :, :FV], tmpc[:, :, :FV],
                                         cf[j - 2][:, :, :FV])
                elif j >= 3:
                    tmpc = tmpp.tile([128, 2, FT], f16, tag="tmpc")
                    nc.vector.tensor_mul(tmpc[:], cf[j - 1][:], twoc[:])
                    nc.vector.tensor_sub(cf[j][:], tmpc[:], cf[j - 2][:])
                # scale u-part by Wa[a]*BJ[j-1] on the Scalar engine
                # (ACT Identity with per-partition scale; keeps DVE free)
                for c in range(2):
                    nc.scalar.activation(
                        us[j][:, :, c * NH:(c + 1) * NH],
                        cf[j][:, :, FV + c * NH:FV + (c + 1) * NH],
                        AF.Identity,
                        scale=biasv_sb[:, 7 + 2 * (j - 1) + c:
                                       8 + 2 * (j - 1) + c],
                    )
                # scoring: sin_u pairs cos_v, cos_u pairs sin_v
                for fn in range(2):
                    for c in range(2):
                        for mb in range(4):
                            nc.tensor.matmul(
                                sc[mb][:],
                                cf[j][:, 1 - fn,
                                      c * N + mb * 128: c * N + (mb + 1) * 128],
                                us[j][:, fn, c * NH:(c + 1) * NH],
                                start=(j == 1 and fn == 0 and c == 0),
                                stop=(j == FJ and fn == 1 and c == 1),
                                skip_group_check=True,
                            )

            attT = attp.tile([128, 4, NH], f16, tag="attT")
            out_sb = opool.tile([128, 2, D], f32, tag="out")
            # reuse the (dead) theta-tile PSUM slots for the final accums
            if bg_zero:
                fos = [gps.tile([128, D], f32, tag="thv", name="fo0", bufs=1),
                       gps.tile([128, D], f32, tag="thu", name="fo1", bufs=1)]
            else:
                fos = [gps.tile([128, D], f32, tag="th", name=f"fo{nb}")
                       for nb in range(2)]
            for mb in range(4):
                nc.scalar.activation(
                    attT[:, mb, :], sc[mb][:], AF.Sigmoid, bias=sgb_sb[:, 0:1]
                )
                for nb in range(2):
                    nc.tensor.matmul(
                        fos[nb][:],
                        attT[:, mb, nb * 128:(nb + 1) * 128],
                        xkT_sb[:, mb, :],
                        start=(mb == 0),
                        stop=(mb == 3),
                        skip_group_check=True,
                    )
            for nb in range(2):
                nc.vector.tensor_copy(out_sb[:, nb, :], fos[nb][:])

            nc.sync.dma_start(
                out.ap().rearrange("(nb p) d -> p nb d", p=128), out_sb[:]
            )

    nc.compile()
    return nc


def _prep_inputs_v2(x, Wg1, Wg2, bg, Wa_w, Wa_b, ba):
    """Host-side packing/slicing only (no reference math)."""
    x = np.asarray(x, np.float32)
    w1s = FS * np.asarray(Wg1, np.float32).T
    w2s = FS * np.asarray(Wg2, np.float32).T
    bgv = FS * np.asarray(bg, np.float32)
    biasv = np.empty((128, 7 + 2 * FJ), np.float32)
    biasv[:, 0:2] = bgv.reshape(2, 128).T
    biasv[:, 2:4] = bgv.reshape(2, 128).T + np.float32(np.pi / 2)
    biasv[:, 4:6] = np.asarray(Wa_w, np.float32).reshape(2, 128).T
    biasv[:, 6] = float(np.asarray(Wa_b).ravel()[0]) \
        + float(np.asarray(ba).ravel()[0])
    wac = np.asarray(Wa_w, np.float32).reshape(2, 128).T
    for j in range(1, FJ + 1):
        for c in range(2):
            biasv[:, 7 + 2 * (j - 1) + c] = wac[:, c] * np.float32(BJ[j - 1])
    in_maps = []
    for c in range(NCORES):
        b, half = c // 2, c % 2
        xb = x[b]
        import os
        dt = np.float32 if int(os.environ.get("K_F32IN", "0")) else np.float16
        vin = np.ascontiguousarray(np.concatenate([w2s, xb], axis=1), dtype=dt)
        uin = np.ascontiguousarray(
            np.concatenate([w1s, xb[:, half * NH:(half + 1) * NH]], axis=1),
            dtype=dt)
        in_maps.append({
            "vin": vin,
            "uin": uin,
            "biasv": np.ascontiguousarray(biasv),
            "xkT": np.ascontiguousarray(xb.T.astype(np.float16)),
        })
    return in_maps


def _run(inputs, trace=False):
    from concourse.bass_utils import run_bass_kernel_spmd

    bg_zero = bool(np.all(np.asarray(inputs["bg"]) == 0))
    key = ("nc", bg_zero)
    if key not in _cache:
        _cache[key] = _build_nc_v2(bg_zero=bg_zero)
    nc = _cache[key]
    in_maps = _prep_inputs_v2(**inputs)
    res = run_bass_kernel_spmd(
        nc, in_maps, core_ids=list(range(NCORES)), trace=trace
    )
    out = np.empty((B, N, D), np.float32)
    for c in range(NCORES):
        b, half = c // 2, c % 2
        out[b, half * NH:(half + 1) * NH] = res.results[c]["out"]
    return out, res


def kernel(**inputs):
    out, _ = _run(inputs, trace=False)
    return out


# revision 27
# speedup vs baseline: 1.3083x; 1.0399x over previous
"""Additive-attention kernel for Trainium2 (8 NeuronCores, SPMD).

Problem (per batch b of B=4):
    xt      = x[b].T                                  # (N=512, D=96)
    g1      = xt @ Wg1.T                              # (512, 256)
    g2      = xt @ Wg2.T                              # (512, 256)
    score   = sum_a Wa[a] * tanh(g1[n,a] + g2[m,a] + bg[a])    # (512, 512)
    att     = sigmoid(score + Wa_b + ba)
    out[b]  = att @ xt                                # (512, 96)

Sharding: core c handles batch b = c//2 and query-rows n in
[(c%2)*256, (c%2)*256+256).  Each core computes its full out rows; the
host concatenates.

Algorithm (v2, Fourier factorization): approximate
    tanh(u+v) ~= sum_{j=1..FJ} BJ[j-1] * sin(j*S*(u+v)),   S = pi/FL
(coefficients from a smoothness-regularized weighted least-squares fit
of tanh on |u+v|<=12 with free periodic completion).  Each harmonic
separates:  sin(jTu+jTv) = sin(jTu)cos(jTv) + cos(jTu)sin(jTv), so the
whole N x N score matrix becomes plain matmuls over a contraction dim
of (a, j, sin|cos) pairs:

  - theta = S*(g + bg) per side via PE matmuls (K=D=96).
  - base features sin(theta), cos(theta) via ACT Sin (args stay within
    the LUT's [-pi, pi] domain: |S*g| + pi/2 < pi for |g| <= FL/2).
  - harmonics via the Chebyshev recurrence f_j = 2cos(theta)*f_{j-1} -
    f_{j-2} on the Vector engine in fp16 (2 tensor_tensor ops per j
    over a combined [128, 2, 1536] tile holding both sides and both
    sin/cos lanes).
  - u-side features scaled by Wa[a]*BJ[j-1] (tensor_scalar, per-
    partition Wa vector + immediate).
  - scoring: per (j, fn, a-chunk, m-block) matmul with the v-side
    feature block as the stationary operand -> scoreT[m, n] accumulates
    into 4 PSUM banks [128, 256] fp32.
  - sigmoid (+Wa_b+ba) PSUM->SBUF fp16 yields attT[m, n] directly, the
    lhsT of the final out[n, d] matmul against x[b].T (fp16).
"""

import numpy as np

B, D, N, A = 4, 96, 512, 256
NH = N // 2          # query rows per core
NCORES = 8

FJ = 11
FL = 12.0
FS = float(np.pi / FL)
BJ = [1.25423644, -0.03484568, 0.37848898, -0.05228383, 0.19638299,
      -0.05586967, 0.10543837, -0.03807847, 0.04397452, -0.0051771,
      0.01922985]

_cache = {}


def _build_nc_v2(bg_zero=False):
    import concourse.bacc as bacc
    import concourse.mybir as mybir
    from concourse import tile

    f32 = mybir.dt.float32
    f16 = mybir.dt.float16
    AF = mybir.ActivationFunctionType
    MULT = mybir.AluOpType.mult

    nc = bacc.Bacc("TRN2", target_bir_lowering=False)

    # packed inputs (fp32: the fp16 variant shifts SBUF tile addresses
    # into a layout that slows DVE tensor_tensor ops by ~20%)
    import os
    _fin = f32 if int(os.environ.get("K_F32IN", "0")) else f16
    vin = nc.dram_tensor("vin", [D, A + N], _fin, kind="ExternalInput")
    uin = nc.dram_tensor("uin", [D, A + NH], _fin, kind="ExternalInput")
    biasv = nc.dram_tensor("biasv", [128, 7 + 2 * FJ], f32, kind="ExternalInput")
    xkT = nc.dram_tensor("xkT", [N, D], f16, kind="ExternalInput")
    out = nc.dram_tensor("out", [NH, D], f32, kind="ExternalOutput")

    with tile.TileContext(nc) as tc:
        with (
            tc.tile_pool(name="consts", bufs=1) as consts,
            tc.tile_pool(name="ufeat", bufs=1) as ufeat,
            tc.tile_pool(name="uscal", bufs=1) as uscal,
            tc.tile_pool(name="tmpp", bufs=2) as tmpp,
            tc.tile_pool(name="gps", bufs=2, space="PSUM") as gps,
            tc.tile_pool(name="scps", bufs=1, space="PSUM") as scps,
            tc.tile_pool(name="attp", bufs=1) as attp,
            tc.tile_pool(name="opool", bufs=1) as opool,
        ):
            vin_sb = consts.tile([D, A + N], _fin, tag="vin")
            uin_sb = consts.tile([D, A + NH], _fin, tag="uin")
            biasv_sb = consts.tile([128, 7 + 2 * FJ], f32, tag="biasv")
            xkT_sb = consts.tile([128, 4, D], f16, tag="xkT")
            w2_sb = vin_sb[:, :A]
            xk_sb = vin_sb[:, A:A + N]
            w1_sb = uin_sb[:, :A]
            xq_sb = uin_sb[:, A:A + NH]
            bsin_sb = biasv_sb[:, 0:2]
            bcos_sb = biasv_sb[:, 2:4]
            wav_sb = biasv_sb[:, 4:6]
            sgb_sb = biasv_sb[:, 6:7]

            # dummy Sin on garbage to preload ACT table sets during DMAs
            dummy = consts.tile([128, 1], f32, tag="dummy")
            nc.gpsimd.memset(dummy[:], 0.0)
            nc.scalar.activation(dummy[:], dummy[:], AF.Sin)

            nc.sync.dma_start(vin_sb[:], vin.ap())
            nc.sync.dma_start(biasv_sb[:], biasv.ap())
            nc.sync.dma_start(uin_sb[:], uin.ap())
            nc.scalar.dma_start(
                xkT_sb[:], xkT.ap().rearrange("(mb p) d -> p mb d", p=128)
            )

            # combined feature tiles, j = 1..FJ:
            # [128, (sin|cos), v-part(c*512+m) | u-part(1024 + c*256+n)]
            FV = N * 2            # 1024: v-part width
            FT = FV + NH * 2      # 1536: total width
            cf = [ufeat.tile([128, 2, FT], f16, tag=f"cf{j}", name=f"cf{j}")
                  if j >= 1 else None for j in range(FJ + 1)]
            us = [uscal.tile([128, 2, NH * 2], f16, tag=f"us{j}", name=f"us{j}")
                  if j >= 1 else None for j in range(FJ + 1)]
            twoc = consts.tile([128, 2, FT], f16, tag="twoc")

            # theta tiles + base features (j=1); v-side first, cos first
            HPI = float(np.pi / 2)
            if bg_zero:
                # bg == 0: immediate biases, chunk-merged theta tiles/sins
                thv = gps.tile([128, FV], f32, tag="thv", name="thv", bufs=1)
                for c in range(2):
                    nc.tensor.matmul(thv[:, c * N:(c + 1) * N],
                                     w2_sb[:, c * 128:(c + 1) * 128], xk_sb[:])
                thu = gps.tile([128, N], f32, tag="thu", name="thu", bufs=1)
                for c in range(2):
                    nc.tensor.matmul(thu[:, c * NH:(c + 1) * NH],
                                     w1_sb[:, c * 128:(c + 1) * 128], xq_sb[:])
                nc.scalar.activation(cf[1][:, 1, :FV], thv[:], AF.Sin,
                                     bias=bcos_sb[:, 0:1])
                nc.scalar.activation(cf[1][:, 0, :FV], thv[:], AF.Sin,
                                     bias=bsin_sb[:, 0:1])
                nc.scalar.activation(cf[1][:, 1, FV:], thu[:], AF.Sin,
                                     bias=bcos_sb[:, 0:1])
                nc.scalar.activation(cf[1][:, 0, FV:], thu[:], AF.Sin,
                                     bias=bsin_sb[:, 0:1])
            else:
                thvs = []
                for c in range(2):
                    thv = gps.tile([128, N], f32, tag="th", name=f"thv{c}")
                    nc.tensor.matmul(thv[:], w2_sb[:, c * 128:(c + 1) * 128],
                                     xk_sb[:])
                    thvs.append(thv)
                thus = []
                for c in range(2):
                    thu = gps.tile([128, N], f32, tag="th", name=f"thu{c}")
                    nc.tensor.matmul(thu[:, :NH],
                                     w1_sb[:, c * 128:(c + 1) * 128], xq_sb[:])
                    thus.append(thu)
                for c in range(2):
                    nc.scalar.activation(cf[1][:, 1, c * N:(c + 1) * N],
                                         thvs[c][:], AF.Sin,
                                         bias=bcos_sb[:, c:c + 1])
                    nc.scalar.activation(cf[1][:, 0, c * N:(c + 1) * N],
                                         thvs[c][:], AF.Sin,
                                         bias=bsin_sb[:, c:c + 1])
                for c in range(2):
                    nc.scalar.activation(
                        cf[1][:, 1, FV + c * NH:FV + (c + 1) * NH],
                        thus[c][:, :NH], AF.Sin, bias=bcos_sb[:, c:c + 1])
                    nc.scalar.activation(
                        cf[1][:, 0, FV + c * NH:FV + (c + 1) * NH],
                        thus[c][:, :NH], AF.Sin, bias=bsin_sb[:, c:c + 1])

            for fn in range(2):
                nc.vector.tensor_scalar_mul(twoc[:, fn, :], cf[1][:, 1, :], 2.0)

            sc = [scps.tile([128, NH], f32, tag=f"sc{mb}", name=f"sc{mb}")
                  for mb in range(4)]

            for j in range(1, FJ + 1):
                if j == 2:
                    # f_2 = 2c*f_1 - f_0 with f_0 = (0, 1)
                    tmpc = tmpp.tile([128, 2, FT], f16, tag="tmpc")
                    nc.vector.tensor_mul(tmpc[:], cf[1][:], twoc[:])
                    nc.vector.tensor_copy(cf[2][:, 0, :], tmpc[:, 0, :])
                    nc.vector.tensor_scalar_add(cf[2][:, 1, :], tmpc[:, 1, :],
                                                -1.0)
                elif j == FJ:
                    # last harmonic: u-part first so the tail can start
                    tmpc = tmpp.tile([128, 2, FT], f16, tag="tmpc")
                    nc.vector.tensor_mul(tmpc[:], cf[j - 1][:], twoc[:])
                    nc.vector.tensor_sub(cf[j][:, :, FV:], tmpc[:, :, FV:],
                                         cf[j - 2][:, :, FV:])
                    nc.vector.tensor_sub(cf[j][:, :, :FV], tmpc[:, :, :FV],
                                         cf[j - 2][:, :, :FV])
                elif j >= 3:
                    tmpc = tmpp.tile([128, 2, FT], f16, tag="tmpc")
                    nc.vector.tensor_mul(tmpc[:], cf[j - 1][:], twoc[:])
                    nc.vector.tensor_sub(cf[j][:], tmpc[:], cf[j - 2][:])
                # scale u-part by Wa[a]*BJ[j-1] on the Scalar engine
                # (ACT Identity with per-partition scale; keeps DVE free)
                for c in range(2):
                    nc.scalar.activation(
                        us[j][:, :, c * NH:(c + 1) * NH],
                        cf[j][:, :, FV + c * NH:FV + (c + 1) * NH],
                        AF.Identity,
                        scale=biasv_sb[:, 7 + 2 * (j - 1) + c:
                                       8 + 2 * (j - 1) + c],
                    )
                # scoring: sin_u pairs cos_v, cos_u pairs sin_v
                for fn in range(2):
                    for c in range(2):
                        for mb in range(4):
                            nc.tensor.matmul(
                                sc[mb][:],
                                cf[j][:, 1 - fn,
                                      c * N + mb * 128: c * N + (mb + 1) * 128],
                                us[j][:, fn, c * NH:(c + 1) * NH],
                                start=(j == 1 and fn == 0 and c == 0),
                                stop=(j == FJ and fn == 1 and c == 1),
                                skip_group_check=True,
                            )

            attT = attp.tile([128, 4, NH], f16, tag="attT")
            out_sb = opool.tile([128, 2, D], f32, tag="out")
            # reuse the (dead) theta-tile PSUM slots for the final accums
            if bg_zero:
                fos = [gps.tile([128, D], f32, tag="thv", name="fo0", bufs=1),
                       gps.tile([128, D], f32, tag="thu", name="fo1", bufs=1)]
            else:
                fos = [gps.tile([128, D], f32, tag="th", name=f"fo{nb}")
                       for nb in range(2)]
            for mb in range(4):
                nc.scalar.activation(
                    attT[:, mb, :], sc[mb][:], AF.Sigmoid, bias=sgb_sb[:, 0:1]
                )
                for nb in range(2):
                    nc.tensor.matmul(
                        fos[nb][:],
                        attT[:, mb, nb * 128:(nb + 1) * 128],
                        xkT_sb[:, mb, :],
                        start=(mb == 0),
                        stop=(mb == 3),
                        skip_group_check=True,
                    )
            for nb in range(2):
                nc.vector.tensor_copy(out_sb[:, nb, :], fos[nb][:])

            nc.sync.dma_start(
                out.ap().rearrange("(nb p) d -> p nb d", p=128), out_sb[:]
            )

    nc.compile()
    return nc


def _prep_inputs_v2(x, Wg1, Wg2, bg, Wa_w, Wa_b, ba):
    """Host-side packing/slicing only (no reference math)."""
    x = np.asarray(x, np.float32)
    w1s = FS * np.asarray(Wg1, np.float32).T
    w2s = FS * np.asarray(Wg2, np.float32).T
    bgv = FS * np.asarray(bg, np.float32)
    biasv = np.empty((128, 7 + 2 * FJ), np.float32)
    biasv[:, 0:2] = bgv.reshape(2, 128).T
    biasv[:, 2:4] = bgv.reshape(2, 128).T + np.float32(np.pi / 2)
    biasv[:, 4:6] = np.asarray(Wa_w, np.float32).reshape(2, 128).T
    biasv[:, 6] = float(np.asarray(Wa_b).ravel()[0]) \
        + float(np.asarray(ba).ravel()[0])
    wac = np.asarray(Wa_w, np.float32).reshape(2, 128).T
    for j in range(1, FJ + 1):
        for c in range(2):
            biasv[:, 7 + 2 * (j - 1) + c] = wac[:, c] * np.float32(BJ[j - 1])
    in_maps = []
    for c in range(NCORES):
        b, half = c // 2, c % 2
        xb = x[b]
        import os
        dt = np.float32 if int(os.environ.get("K_F32IN", "0")) else np.float16
        vin = np.ascontiguousarray(np.concatenate([w2s, xb], axis=1), dtype=dt)
        uin = np.ascontiguousarray(
            np.concatenate([w1s, xb[:, half * NH:(half + 1) * NH]], axis=1),
            dtype=dt)
        in_maps.append({
            "vin": vin,
            "uin": uin,
            "biasv": np.ascontiguousarray(biasv),
            "xkT": np.ascontiguousarray(xb.T.astype(np.float16)),
        })
    return in_maps


def _run(inputs, trace=False):
    from concourse.bass_utils import run_bass_kernel_spmd

    bg_zero = bool(np.all(np.asarray(inputs["bg"]) == 0))
    key = ("nc", bg_zero)
    if key not in _cache:
        _cache[key] = _build_nc_v2(bg_zero=bg_zero)
    nc = _cache[key]
    in_maps = _prep_inputs_v2(**inputs)
    res = run_bass_kernel_spmd(
        nc, in_maps, core_ids=list(range(NCORES)), trace=trace
    )
    out = np.empty((B, N, D), np.float32)
    for c in range(NCORES):
        b, half = c // 2, c % 2
        out[b, half * NH:(half + 1) * NH] = res.results[c]["out"]
    return out, res


def kernel(**inputs):
    out, _ = _run(inputs, trace=False)
    return out


# revision 28
# speedup vs baseline: 1.3132x; 1.0037x over previous
"""Additive-attention kernel for Trainium2 (8 NeuronCores, SPMD).

Problem (per batch b of B=4):
    xt      = x[b].T                                  # (N=512, D=96)
    g1      = xt @ Wg1.T                              # (512, 256)
    g2      = xt @ Wg2.T                              # (512, 256)
    score   = sum_a Wa[a] * tanh(g1[n,a] + g2[m,a] + bg[a])    # (512, 512)
    att     = sigmoid(score + Wa_b + ba)
    out[b]  = att @ xt                                # (512, 96)

Sharding: core c handles batch b = c//2 and query-rows n in
[(c%2)*256, (c%2)*256+256).  Each core computes its full out rows; the
host concatenates.

Algorithm (v2, Fourier factorization): approximate
    tanh(u+v) ~= sum_{j=1..FJ} BJ[j-1] * sin(j*S*(u+v)),   S = pi/FL
(coefficients from a smoothness-regularized weighted least-squares fit
of tanh on |u+v|<=12 with free periodic completion).  Each harmonic
separates:  sin(jTu+jTv) = sin(jTu)cos(jTv) + cos(jTu)sin(jTv), so the
whole N x N score matrix becomes plain matmuls over a contraction dim
of (a, j, sin|cos) pairs:

  - theta = S*(g + bg) per side via PE matmuls (K=D=96).
  - base features sin(theta), cos(theta) via ACT Sin (args stay within
    the LUT's [-pi, pi] domain: |S*g| + pi/2 < pi for |g| <= FL/2).
  - harmonics via the Chebyshev recurrence f_j = 2cos(theta)*f_{j-1} -
    f_{j-2} on the Vector engine in fp16 (2 tensor_tensor ops per j
    over a combined [128, 2, 1536] tile holding both sides (v: all 512
    keys x 2 a-chunks; u: own 256 queries x 2) and both sin/cos lanes).
  - u-side features scaled by Wa[a]*BJ[j-1] (tensor_scalar, per-
    partition Wa vector + immediate).
  - scoring: per (j, fn, a-chunk, m-block) matmul with the v-side
    feature block as the stationary operand -> scoreT[m, n] accumulates
    into 4 PSUM banks [128, 256] fp32.
  - sigmoid (+Wa_b+ba) PSUM->SBUF fp16 yields attT[m, n] directly, the
    lhsT of the final out[n, d] matmul against x[b].T (fp16).
"""

import numpy as np

B, D, N, A = 4, 96, 512, 256
NH = N // 2          # query rows per core
NCORES = 8

FJ = 11
FL = 12.0
FS = float(np.pi / FL)
BJ = [1.25423644, -0.03484568, 0.37848898, -0.05228383, 0.19638299,
      -0.05586967, 0.10543837, -0.03807847, 0.04397452, -0.0051771,
      0.01922985]

_cache = {}


def _build_nc_v2(bg_zero=False):
    import concourse.bacc as bacc
    import concourse.mybir as mybir
    from concourse import tile

    f32 = mybir.dt.float32
    f16 = mybir.dt.float16
    AF = mybir.ActivationFunctionType
    MULT = mybir.AluOpType.mult

    nc = bacc.Bacc("TRN2", target_bir_lowering=False)

    # packed inputs (fp32: the fp16 variant shifts SBUF tile addresses
    # into a layout that slows DVE tensor_tensor ops by ~20%)
    import os
    _fin = f32 if int(os.environ.get("K_F32IN", "0")) else f16
    vin = nc.dram_tensor("vin", [D, A + N], _fin, kind="ExternalInput")
    uin = nc.dram_tensor("uin", [D, A + NH], _fin, kind="ExternalInput")
    biasv = nc.dram_tensor("biasv", [128, 7 + 2 * FJ], f32, kind="ExternalInput")
    xkT = nc.dram_tensor("xkT", [N, D], f16, kind="ExternalInput")
    out = nc.dram_tensor("out", [NH, D], f32, kind="ExternalOutput")

    with tile.TileContext(nc) as tc:
        with (
            tc.tile_pool(name="consts", bufs=1) as consts,
            tc.tile_pool(name="ufeat", bufs=1) as ufeat,
            tc.tile_pool(name="uscal", bufs=1) as uscal,
            tc.tile_pool(name="tmpp", bufs=2) as tmpp,
            tc.tile_pool(name="gps", bufs=2, space="PSUM") as gps,
            tc.tile_pool(name="scps", bufs=1, space="PSUM") as scps,
            tc.tile_pool(name="attp", bufs=1) as attp,
            tc.tile_pool(name="opool", bufs=1) as opool,
        ):
            vin_sb = consts.tile([D, A + N], _fin, tag="vin")
            uin_sb = consts.tile([D, A + NH], _fin, tag="uin")
            biasv_sb = consts.tile([128, 7 + 2 * FJ], f32, tag="biasv")
            xkT_sb = consts.tile([128, 4, D], f16, tag="xkT")
            w2_sb = vin_sb[:, :A]
            xk_sb = vin_sb[:, A:A + N]
            w1_sb = uin_sb[:, :A]
            xq_sb = uin_sb[:, A:A + NH]
            bsin_sb = biasv_sb[:, 0:2]
            bcos_sb = biasv_sb[:, 2:4]
            wav_sb = biasv_sb[:, 4:6]
            sgb_sb = biasv_sb[:, 6:7]

            # dummy Sin on garbage to preload ACT table sets during DMAs
            dummy = consts.tile([128, 1], f32, tag="dummy")
            nc.gpsimd.memset(dummy[:], 0.0)
            nc.scalar.activation(dummy[:], dummy[:], AF.Sin)

            nc.sync.dma_start(vin_sb[:], vin.ap())
            nc.sync.dma_start(biasv_sb[:], biasv.ap())
            nc.sync.dma_start(uin_sb[:], uin.ap())
            nc.scalar.dma_start(
                xkT_sb[:], xkT.ap().rearrange("(mb p) d -> p mb d", p=128)
            )

            # combined feature tiles, j = 1..FJ:
            # [128, (sin|cos), v-part(c*512+m) | u-part(1024 + c*256+n)]
            FV = N * 2            # 1024: v-part width
            FT = FV + NH * 2      # 1536: total width
            cf = [ufeat.tile([128, 2, FT], f16, tag=f"cf{j}", name=f"cf{j}")
                  if j >= 1 else None for j in range(FJ + 1)]
            us = [uscal.tile([128, 2, NH * 2], f16, tag=f"us{j}", name=f"us{j}")
                  if j >= 1 else None for j in range(FJ + 1)]
            twoc = consts.tile([128, 2, FT], f16, tag="twoc")

            # theta tiles + base features (j=1); v-side first, cos first
            HPI = float(np.pi / 2)
            if bg_zero:
                # bg == 0: immediate biases, chunk-merged theta tiles/sins
                thv = gps.tile([128, FV], f32, tag="thv", name="thv", bufs=1)
                for c in range(2):
                    nc.tensor.matmul(thv[:, c * N:(c + 1) * N],
                                     w2_sb[:, c * 128:(c + 1) * 128], xk_sb[:])
                thu = gps.tile([128, N], f32, tag="thu", name="thu", bufs=1)
                for c in range(2):
                    nc.tensor.matmul(thu[:, c * NH:(c + 1) * NH],
                                     w1_sb[:, c * 128:(c + 1) * 128], xq_sb[:])
                nc.scalar.activation(cf[1][:, 1, :FV], thv[:], AF.Sin,
                                     bias=bcos_sb[:, 0:1])
                nc.scalar.activation(cf[1][:, 0, :FV], thv[:], AF.Sin,
                                     bias=bsin_sb[:, 0:1])
                nc.scalar.activation(cf[1][:, 1, FV:], thu[:], AF.Sin,
                                     bias=bcos_sb[:, 0:1])
                nc.scalar.activation(cf[1][:, 0, FV:], thu[:], AF.Sin,
                                     bias=bsin_sb[:, 0:1])
            else:
                thvs = []
                for c in range(2):
                    thv = gps.tile([128, N], f32, tag="th", name=f"thv{c}")
                    nc.tensor.matmul(thv[:], w2_sb[:, c * 128:(c + 1) * 128],
                                     xk_sb[:])
                    thvs.append(thv)
                thus = []
                for c in range(2):
                    thu = gps.tile([128, N], f32, tag="th", name=f"thu{c}")
                    nc.tensor.matmul(thu[:, :NH],
                                     w1_sb[:, c * 128:(c + 1) * 128], xq_sb[:])
                    thus.append(thu)
                for c in range(2):
                    nc.scalar.activation(cf[1][:, 1, c * N:(c + 1) * N],
                                         thvs[c][:], AF.Sin,
                                         bias=bcos_sb[:, c:c + 1])
                    nc.scalar.activation(cf[1][:, 0, c * N:(c + 1) * N],
                                         thvs[c][:], AF.Sin,
                                         bias=bsin_sb[:, c:c + 1])
                for c in range(2):
                    nc.scalar.activation(
                        cf[1][:, 1, FV + c * NH:FV + (c + 1) * NH],
                        thus[c][:, :NH], AF.Sin, bias=bcos_sb[:, c:c + 1])
                    nc.scalar.activation(
                        cf[1][:, 0, FV + c * NH:FV + (c + 1) * NH],
                        thus[c][:, :NH], AF.Sin, bias=bsin_sb[:, c:c + 1])

            for fn in range(2):
                nc.vector.tensor_scalar_mul(twoc[:, fn, :], cf[1][:, 1, :], 2.0)

            sc = [scps.tile([128, NH], f32, tag=f"sc{mb}", name=f"sc{mb}")
                  for mb in range(4)]

            for j in range(1, FJ + 1):
                if j == 2:
                    # f_2 = 2c*f_1 - f_0 with f_0 = (0, 1)
                    tmpc = tmpp.tile([128, 2, FT], f16, tag="tmpc")
                    nc.vector.tensor_mul(tmpc[:], cf[1][:], twoc[:])
                    nc.vector.tensor_copy(cf[2][:, 0, :], tmpc[:, 0, :])
                    nc.vector.tensor_scalar_add(cf[2][:, 1, :], tmpc[:, 1, :],
                                                -1.0)
                elif j == FJ:
                    # last harmonic: u-part first so the tail can start
                    tmpc = tmpp.tile([128, 2, FT], f16, tag="tmpc")
                    nc.vector.tensor_mul(tmpc[:], cf[j - 1][:], twoc[:])
                    nc.vector.tensor_sub(cf[j][:, :, FV:], tmpc[:, :, FV:],
                                         cf[j - 2][:, :, FV:])
                    nc.vector.tensor_sub(cf[j][:, :, :FV], tmpc[:, :, :FV],
                                         cf[j - 2][:, :, :FV])
                elif j >= 3:
                    tmpc = tmpp.tile([128, 2, FT], f16, tag="tmpc")
                    nc.vector.tensor_mul(tmpc[:], cf[j - 1][:], twoc[:])
                    nc.vector.tensor_sub(cf[j][:], tmpc[:], cf[j - 2][:])
                # scale u-part by Wa[a]*BJ[j-1] on the Scalar engine
                # (ACT Identity with per-partition scale; keeps DVE free)
                for c in range(2):
                    nc.scalar.activation(
                        us[j][:, :, c * NH:(c + 1) * NH],
                        cf[j][:, :, FV + c * NH:FV + (c + 1) * NH],
                        AF.Identity,
                        scale=biasv_sb[:, 7 + 2 * (j - 1) + c:
                                       8 + 2 * (j - 1) + c],
                    )
                # scoring: sin_u pairs cos_v, cos_u pairs sin_v
                for fn in range(2):
                    for c in range(2):
                        for mb in range(4):
                            nc.tensor.matmul(
                                sc[mb][:],
                                cf[j][:, 1 - fn,
                                      c * N + mb * 128: c * N + (mb + 1) * 128],
                                us[j][:, fn, c * NH:(c + 1) * NH],
                                start=(j == 1 and fn == 0 and c == 0),
                                stop=(j == FJ and fn == 1 and c == 1),
                                skip_group_check=True,
                            )

            attT = attp.tile([128, 4, NH], f16, tag="attT")
            out_sb = opool.tile([128, 2, D], f32, tag="out")
            # reuse the (dead) theta-tile PSUM slots for the final accums
            if bg_zero:
                fos = [gps.tile([128, D], f32, tag="thv", name="fo0", bufs=1),
                       gps.tile([128, D], f32, tag="thu", name="fo1", bufs=1)]
            else:
                fos = [gps.tile([128, D], f32, tag="th", name=f"fo{nb}")
                       for nb in range(2)]
            for mb in range(4):
                nc.scalar.activation(
                    attT[:, mb, :], sc[mb][:], AF.Sigmoid, bias=sgb_sb[:, 0:1]
                )
                for nb in range(2):
                    nc.tensor.matmul(
                        fos[nb][:],
                        attT[:, mb, nb * 128:(nb + 1) * 128],
                        xkT_sb[:, mb, :],
                        start=(mb == 0),
                        stop=(mb == 3),
                        skip_group_check=True,
                    )
            for nb in range(2):
                nc.vector.tensor_copy(out_sb[:, nb, :], fos[nb][:])

            nc.sync.dma_start(
                out.ap().rearrange("(nb p) d -> p nb d", p=128), out_sb[:]
            )

    nc.compile()
    return nc


def _prep_inputs_v2(x, Wg1, Wg2, bg, Wa_w, Wa_b, ba):
    """Host-side packing/slicing only (no reference math)."""
    x = np.asarray(x, np.float32)
    w1s = FS * np.asarray(Wg1, np.float32).T
    w2s = FS * np.asarray(Wg2, np.float32).T
    bgv = FS * np.asarray(bg, np.float32)
    biasv = np.empty((128, 7 + 2 * FJ), np.float32)
    biasv[:, 0:2] = bgv.reshape(2, 128).T
    biasv[:, 2:4] = bgv.reshape(2, 128).T + np.float32(np.pi / 2)
    biasv[:, 4:6] = np.asarray(Wa_w, np.float32).reshape(2, 128).T
    biasv[:, 6] = float(np.asarray(Wa_b).ravel()[0]) \
        + float(np.asarray(ba).ravel()[0])
    wac = np.asarray(Wa_w, np.float32).reshape(2, 128).T
    for j in range(1, FJ + 1):
        for c in range(2):
            biasv[:, 7 + 2 * (j - 1) + c] = wac[:, c] * np.float32(BJ[j - 1])
    in_maps = []
    for c in range(NCORES):
        b, half = c // 2, c % 2
        xb = x[b]
        import os
        dt = np.float32 if int(os.environ.get("K_F32IN", "0")) else np.float16
        vin = np.ascontiguousarray(np.concatenate([w2s, xb], axis=1), dtype=dt)
        uin = np.ascontiguousarray(
            np.concatenate([w1s, xb[:, half * NH:(half + 1) * NH]], axis=1),
            dtype=dt)
        in_maps.append({
            "vin": vin,
            "uin": uin,
            "biasv": np.ascontiguousarray(biasv),
            "xkT": np.ascontiguousarray(xb.T.astype(np.float16)),
        })
    return in_maps


def _run(inputs, trace=False):
    from concourse.bass_utils import run_bass_kernel_spmd

    bg_zero = bool(np.all(np.asarray(inputs["bg"]) == 0))
    key = ("nc", bg_zero)
    if key not in _cache:
        _cache[key] = _build_nc_v2(bg_zero=bg_zero)
    nc = _cache[key]
    in_maps = _prep_inputs_v2(**inputs)
    res = run_bass_kernel_spmd(
        nc, in_maps, core_ids=list(range(NCORES)), trace=trace
    )
    out = np.empty((B, N, D), np.float32)
    for c in range(NCORES):
        b, half = c // 2, c % 2
        out[b, half * NH:(half + 1) * NH] = res.results[c]["out"]
    return out, res


def kernel(**inputs):
    out, _ = _run(inputs, trace=False)
    return out


# revision 29
# speedup vs baseline: 1.4644x; 1.1152x over previous
"""Additive-attention kernel for Trainium2 (8 NeuronCores, SPMD).

Problem (per batch b of B=4):
    xt      = x[b].T                                  # (N=512, D=96)
    g1      = xt @ Wg1.T                              # (512, 256)
    g2      = xt @ Wg2.T                              # (512, 256)
    score   = sum_a Wa[a] * tanh(g1[n,a] + g2[m,a] + bg[a])    # (512, 512)
    att     = sigmoid(score + Wa_b + ba)
    out[b]  = att @ xt                                # (512, 96)

Sharding: core c handles batch b = c//2 and query-rows n in
[(c%2)*256, (c%2)*256+256).  Each core computes its full out rows; the
host concatenates.

Algorithm (v2, Fourier factorization): approximate
    tanh(u+v) ~= sum_{j=1..FJ} BJ[j-1] * sin(j*S*(u+v)),   S = pi/FL
(coefficients from a smoothness-regularized weighted least-squares fit
of tanh on |u+v|<=12 with free periodic completion).  Each harmonic
separates:  sin(jTu+jTv) = sin(jTu)cos(jTv) + cos(jTu)sin(jTv), so the
whole N x N score matrix becomes plain matmuls over a contraction dim
of (a, j, sin|cos) pairs:

  - theta = S*(g + bg) per side via PE matmuls (K=D=96).
  - base features sin(theta), cos(theta) via ACT Sin (args stay within
    the LUT's [-pi, pi] domain: |S*g| + pi/2 < pi for |g| <= FL/2).
  - harmonics via the Chebyshev recurrence f_j = 2cos(theta)*f_{j-1} -
    f_{j-2} on the Vector engine in fp16 (2 tensor_tensor ops per j
    over a combined [128, 2, 1536] tile holding both sides (v: all 512
    keys x 2 a-chunks; u: own 256 queries x 2) and both sin/cos lanes).
  - u-side features scaled by Wa[a]*BJ[j-1] (tensor_scalar, per-
    partition Wa vector + immediate).
  - scoring: per (j, fn, a-chunk, m-block) matmul with the v-side
    feature block as the stationary operand -> scoreT[m, n] accumulates
    into 4 PSUM banks [128, 256] fp32.
  - sigmoid (+Wa_b+ba) PSUM->SBUF fp16 yields attT[m, n] directly, the
    lhsT of the final out[n, d] matmul against x[b].T (fp16).
"""

import numpy as np

B, D, N, A = 4, 96, 512, 256
NH = N // 2          # query rows per core
NCORES = 8

FJ = 10
FL = 11.5
FS = float(np.pi / FL)
BJ = [1.24406304, -0.02205928, 0.3522805, -0.0231798, 0.15566014,
      -0.01755559, 0.05501432, -0.00135519, 0.01892349, 0.01975335]

_cache = {}


def _build_nc_v2(bg_zero=False):
    import concourse.bacc as bacc
    import concourse.mybir as mybir
    from concourse import tile

    f32 = mybir.dt.float32
    f16 = mybir.dt.float16
    AF = mybir.ActivationFunctionType
    MULT = mybir.AluOpType.mult

    nc = bacc.Bacc("TRN2", target_bir_lowering=False)

    # packed inputs (fp32: the fp16 variant shifts SBUF tile addresses
    # into a layout that slows DVE tensor_tensor ops by ~20%)
    import os
    _fin = f32 if int(os.environ.get("K_F32IN", "0")) else f16
    vin = nc.dram_tensor("vin", [D, A + N], _fin, kind="ExternalInput")
    uin = nc.dram_tensor("uin", [D, A + NH], _fin, kind="ExternalInput")
    biasv = nc.dram_tensor("biasv", [128, 7 + 2 * FJ], f32, kind="ExternalInput")
    xkT = nc.dram_tensor("xkT", [N, D], f16, kind="ExternalInput")
    out = nc.dram_tensor("out", [NH, D], f32, kind="ExternalOutput")

    with tile.TileContext(nc) as tc:
        with (
            tc.tile_pool(name="consts", bufs=1) as consts,
            tc.tile_pool(name="ufeat", bufs=1) as ufeat,
            tc.tile_pool(name="uscal", bufs=1) as uscal,
            tc.tile_pool(name="tmpp", bufs=2) as tmpp,
            tc.tile_pool(name="gps", bufs=2, space="PSUM") as gps,
            tc.tile_pool(name="scps", bufs=1, space="PSUM") as scps,
            tc.tile_pool(name="attp", bufs=1) as attp,
            tc.tile_pool(name="opool", bufs=1) as opool,
        ):
            vin_sb = consts.tile([D, A + N], _fin, tag="vin")
            uin_sb = consts.tile([D, A + NH], _fin, tag="uin")
            biasv_sb = consts.tile([128, 7 + 2 * FJ], f32, tag="biasv")
            xkT_sb = consts.tile([128, 4, D], f16, tag="xkT")
            w2_sb = vin_sb[:, :A]
            xk_sb = vin_sb[:, A:A + N]
            w1_sb = uin_sb[:, :A]
            xq_sb = uin_sb[:, A:A + NH]
            bsin_sb = biasv_sb[:, 0:2]
            bcos_sb = biasv_sb[:, 2:4]
            wav_sb = biasv_sb[:, 4:6]
            sgb_sb = biasv_sb[:, 6:7]

            # dummy Sin on garbage to preload ACT table sets during DMAs
            dummy = consts.tile([128, 1], f32, tag="dummy")
            nc.gpsimd.memset(dummy[:], 0.0)
            nc.scalar.activation(dummy[:], dummy[:], AF.Sin)

            nc.sync.dma_start(vin_sb[:], vin.ap())
            nc.sync.dma_start(biasv_sb[:], biasv.ap())
            nc.sync.dma_start(uin_sb[:], uin.ap())
            nc.scalar.dma_start(
                xkT_sb[:], xkT.ap().rearrange("(mb p) d -> p mb d", p=128)
            )

            # combined feature tiles, j = 1..FJ:
            # [128, (sin|cos), v-part(c*512+m) | u-part(1024 + c*256+n)]
            FV = N * 2            # 1024: v-part width
            FT = FV + NH * 2      # 1536: total width
            cf = [ufeat.tile([128, 2, FT], f16, tag=f"cf{j}", name=f"cf{j}")
                  if j >= 1 else None for j in range(FJ + 1)]
            us = [uscal.tile([128, 2, NH * 2], f16, tag=f"us{j}", name=f"us{j}")
                  if j >= 1 else None for j in range(FJ + 1)]
            twoc = consts.tile([128, 2, FT], f16, tag="twoc")

            # theta tiles + base features (j=1); v-side first, cos first
            HPI = float(np.pi / 2)
            if bg_zero:
                # bg == 0: immediate biases, chunk-merged theta tiles/sins
                thv = gps.tile([128, FV], f32, tag="thv", name="thv", bufs=1)
                for c in range(2):
                    nc.tensor.matmul(thv[:, c * N:(c + 1) * N],
                                     w2_sb[:, c * 128:(c + 1) * 128], xk_sb[:])
                thu = gps.tile([128, N], f32, tag="thu", name="thu", bufs=1)
                for c in range(2):
                    nc.tensor.matmul(thu[:, c * NH:(c + 1) * NH],
                                     w1_sb[:, c * 128:(c + 1) * 128], xq_sb[:])
                nc.scalar.activation(cf[1][:, 1, :FV], thv[:], AF.Sin,
                                     bias=bcos_sb[:, 0:1])
                nc.scalar.activation(cf[1][:, 1, FV:], thu[:], AF.Sin,
                                     bias=bcos_sb[:, 0:1])
                nc.scalar.activation(cf[1][:, 0, :FV], thv[:], AF.Sin,
                                     bias=bsin_sb[:, 0:1])
                nc.scalar.activation(cf[1][:, 0, FV:], thu[:], AF.Sin,
                                     bias=bsin_sb[:, 0:1])
            else:
                thvs = []
                for c in range(2):
                    thv = gps.tile([128, N], f32, tag="th", name=f"thv{c}")
                    nc.tensor.matmul(thv[:], w2_sb[:, c * 128:(c + 1) * 128],
                                     xk_sb[:])
                    thvs.append(thv)
                thus = []
                for c in range(2):
                    thu = gps.tile([128, N], f32, tag="th", name=f"thu{c}")
                    nc.tensor.matmul(thu[:, :NH],
                                     w1_sb[:, c * 128:(c + 1) * 128], xq_sb[:])
                    thus.append(thu)
                for c in range(2):
                    nc.scalar.activation(cf[1][:, 1, c * N:(c + 1) * N],
                                         thvs[c][:], AF.Sin,
                                         bias=bcos_sb[:, c:c + 1])
                    nc.scalar.activation(cf[1][:, 0, c * N:(c + 1) * N],
                                         thvs[c][:], AF.Sin,
                                         bias=bsin_sb[:, c:c + 1])
                for c in range(2):
                    nc.scalar.activation(
                        cf[1][:, 1, FV + c * NH:FV + (c + 1) * NH],
                        thus[c][:, :NH], AF.Sin, bias=bcos_sb[:, c:c + 1])
                    nc.scalar.activation(
                        cf[1][:, 0, FV + c * NH:FV + (c + 1) * NH],
                        thus[c][:, :NH], AF.Sin, bias=bsin_sb[:, c:c + 1])

            for fn in range(2):
                nc.vector.tensor_scalar_mul(twoc[:, fn, :], cf[1][:, 1, :], 2.0)

            sc = [scps.tile([128, NH], f32, tag=f"sc{mb}", name=f"sc{mb}")
                  for mb in range(4)]

            for j in range(1, FJ + 1):
                if j == 2:
                    # f_2 = 2c*f_1 - f_0 with f_0 = (0, 1):
                    # cos_2 first (only needs the cos lane), sin_2 direct
                    tmpc = tmpp.tile([128, 2, FT], f16, tag="tmpc")
                    nc.vector.tensor_mul(tmpc[:, 1, :], cf[1][:, 1, :],
                                         twoc[:, 1, :])
                    nc.vector.tensor_scalar_add(cf[2][:, 1, :], tmpc[:, 1, :],
                                                -1.0)
                    nc.vector.tensor_mul(cf[2][:, 0, :], cf[1][:, 0, :],
                                         twoc[:, 0, :])
                elif j == FJ:
                    # last harmonic: u-part first so the tail can start
                    tmpc = tmpp.tile([128, 2, FT], f16, tag="tmpc")
                    nc.vector.tensor_mul(tmpc[:], cf[j - 1][:], twoc[:])
                    nc.vector.tensor_sub(cf[j][:, :, FV:], tmpc[:, :, FV:],
                                         cf[j - 2][:, :, FV:])
                    nc.vector.tensor_sub(cf[j][:, :, :FV], tmpc[:, :, :FV],
                                         cf[j - 2][:, :, :FV])
                elif j >= 3:
                    tmpc = tmpp.tile([128, 2, FT], f16, tag="tmpc")
                    nc.vector.tensor_mul(tmpc[:], cf[j - 1][:], twoc[:])
                    nc.vector.tensor_sub(cf[j][:], tmpc[:], cf[j - 2][:])
                # scale u-part by Wa[a]*BJ[j-1] on the Scalar engine
                # (ACT Identity with per-partition scale; keeps DVE free)
                for c in range(2):
                    nc.scalar.activation(
                        us[j][:, :, c * NH:(c + 1) * NH],
                        cf[j][:, :, FV + c * NH:FV + (c + 1) * NH],
                        AF.Identity,
                        scale=biasv_sb[:, 7 + 2 * (j - 1) + c:
                                       8 + 2 * (j - 1) + c],
                    )
                # scoring: sin_u pairs cos_v, cos_u pairs sin_v
                for fn in range(2):
                    for c in range(2):
                        for mb in range(4):
                            nc.tensor.matmul(
                                sc[mb][:],
                                cf[j][:, 1 - fn,
                                      c * N + mb * 128: c * N + (mb + 1) * 128],
                                us[j][:, fn, c * NH:(c + 1) * NH],
                                start=(j == 1 and fn == 0 and c == 0),
                                stop=(j == FJ and fn == 1 and c == 1),
                                skip_group_check=True,
                            )

            attT = attp.tile([128, 4, NH], f16, tag="attT")
            out_sb = opool.tile([128, 2, D], f32, tag="out")
            # reuse the (dead) theta-tile PSUM slots for the final accums
            if bg_zero:
                fos = [gps.tile([128, D], f32, tag="thv", name="fo0", bufs=1),
                       gps.tile([128, D], f32, tag="thu", name="fo1", bufs=1)]
            else:
                fos = [gps.tile([128, D], f32, tag="th", name=f"fo{nb}")
                       for nb in range(2)]
            for mb in range(4):
                nc.scalar.activation(
                    attT[:, mb, :], sc[mb][:], AF.Sigmoid, bias=sgb_sb[:, 0:1]
                )
                for nb in range(2):
                    nc.tensor.matmul(
                        fos[nb][:],
                        attT[:, mb, nb * 128:(nb + 1) * 128],
                        xkT_sb[:, mb, :],
                        start=(mb == 0),
                        stop=(mb == 3),
                        skip_group_check=True,
                    )
            for nb in range(2):
                nc.vector.tensor_copy(out_sb[:, nb, :], fos[nb][:])

            nc.sync.dma_start(
                out.ap().rearrange("(nb p) d -> p nb d", p=128), out_sb[:]
            )

    nc.compile()
    return nc


def _prep_inputs_v2(x, Wg1, Wg2, bg, Wa_w, Wa_b, ba):
    """Host-side packing/slicing only (no reference math)."""
    x = np.asarray(x, np.float32)
    w1s = FS * np.asarray(Wg1, np.float32).T
    w2s = FS * np.asarray(Wg2, np.float32).T
    bgv = FS * np.asarray(bg, np.float32)
    biasv = np.empty((128, 7 + 2 * FJ), np.float32)
    biasv[:, 0:2] = bgv.reshape(2, 128).T
    biasv[:, 2:4] = bgv.reshape(2, 128).T + np.float32(np.pi / 2)
    biasv[:, 4:6] = np.asarray(Wa_w, np.float32).reshape(2, 128).T
    biasv[:, 6] = float(np.asarray(Wa_b).ravel()[0]) \
        + float(np.asarray(ba).ravel()[0])
    wac = np.asarray(Wa_w, np.float32).reshape(2, 128).T
    for j in range(1, FJ + 1):
        for c in range(2):
            biasv[:, 7 + 2 * (j - 1) + c] = wac[:, c] * np.float32(BJ[j - 1])
    in_maps = []
    for c in range(NCORES):
        b, half = c // 2, c % 2
        xb = x[b]
        import os
        dt = np.float32 if int(os.environ.get("K_F32IN", "0")) else np.float16
        vin = np.ascontiguousarray(np.concatenate([w2s, xb], axis=1), dtype=dt)
        uin = np.ascontiguousarray(
            np.concatenate([w1s, xb[:, half * NH:(half + 1) * NH]], axis=1),
            dtype=dt)
        in_maps.append({
            "vin": vin,
            "uin": uin,
            "biasv": np.ascontiguousarray(biasv),
            "xkT": np.ascontiguousarray(xb.T.astype(np.float16)),
        })
    return in_maps


def _run(inputs, trace=False):
    from concourse.bass_utils import run_bass_kernel_spmd

    bg_zero = bool(np.all(np.asarray(inputs["bg"]) == 0))
    key = ("nc", bg_zero)
    if key not in _cache:
        _cache[key] = _build_nc_v2(bg_zero=bg_zero)
    nc = _cache[key]
    in_maps = _prep_inputs_v2(**inputs)
    res = run_bass_kernel_spmd(
        nc, in_maps, core_ids=list(range(NCORES)), trace=trace
    )
    out = np.empty((B, N, D), np.float32)
    for c in range(NCORES):
        b, half = c // 2, c % 2
        out[b, half * NH:(half + 1) * NH] = res.results[c]["out"]
    return out, res


def kernel(**inputs):
    out, _ = _run(inputs, trace=False)
    return out


# revision 30
# speedup vs baseline: 1.4663x; 1.0013x over previous
"""Additive-attention kernel for Trainium2 (8 NeuronCores, SPMD).

Problem (per batch b of B=4):
    xt      = x[b].T                                  # (N=512, D=96)
    g1      = xt @ Wg1.T                              # (512, 256)
    g2      = xt @ Wg2.T                              # (512, 256)
    score   = sum_a Wa[a] * tanh(g1[n,a] + g2[m,a] + bg[a])    # (512, 512)
    att     = sigmoid(score + Wa_b + ba)
    out[b]  = att @ xt                                # (512, 96)

Sharding: core c handles batch b = c//2 and query-rows n in
[(c%2)*256, (c%2)*256+256).  Each core computes its full out rows; the
host concatenates.

Algorithm (v2, Fourier factorization): approximate
    tanh(u+v) ~= sum_{j=1..FJ} BJ[j-1] * sin(j*S*(u+v)),   S = pi/FL
(coefficients from a smoothness-regularized weighted least-squares fit
of tanh on |u+v|<=12 with free periodic completion).  Each harmonic
separates:  sin(jTu+jTv) = sin(jTu)cos(jTv) + cos(jTu)sin(jTv), so the
whole N x N score matrix becomes plain matmuls over a contraction dim
of (a, j, sin|cos) pairs:

  - theta = S*(g + bg) per side via PE matmuls (K=D=96).
  - base features sin(theta), cos(theta) via ACT Sin (args stay within
    the LUT's [-pi, pi] domain: |S*g| + pi/2 < pi for |g| <= FL/2).
  - harmonics via the Chebyshev recurrence f_j = 2cos(theta)*f_{j-1} -
    f_{j-2} on the Vector engine in fp16 (2 tensor_tensor ops per j
    over a combined [128, 2, 1536] tile holding both sides (v: all 512
    keys x 2 a-chunks; u: own 256 queries x 2) and both sin/cos lanes).
  - u-side features scaled by Wa[a]*BJ[j-1] (tensor_scalar, per-
    partition Wa vector + immediate).
  - scoring: per (j, fn, a-chunk, m-block) matmul with the v-side
    feature block as the stationary operand -> scoreT[m, n] accumulates
    into 4 PSUM banks [128, 256] fp32.
  - sigmoid (+Wa_b+ba) PSUM->SBUF fp16 yields attT[m, n] directly, the
    lhsT of the final out[n, d] matmul against x[b].T (fp16).
"""

import numpy as np

B, D, N, A = 4, 96, 512, 256
NH = N // 2          # query rows per core
NCORES = 8

FJ = 10
FL = 11.5
FS = float(np.pi / FL)
BJ = [1.24406304, -0.02205928, 0.3522805, -0.0231798, 0.15566014,
      -0.01755559, 0.05501432, -0.00135519, 0.01892349, 0.01975335]

_cache = {}


def _build_nc_v2(bg_zero=False):
    import concourse.bacc as bacc
    import concourse.mybir as mybir
    from concourse import tile

    f32 = mybir.dt.float32
    f16 = mybir.dt.float16
    AF = mybir.ActivationFunctionType
    MULT = mybir.AluOpType.mult

    nc = bacc.Bacc("TRN2", target_bir_lowering=False)

    # packed inputs (fp32: the fp16 variant shifts SBUF tile addresses
    # into a layout that slows DVE tensor_tensor ops by ~20%)
    import os
    _fin = f32 if int(os.environ.get("K_F32IN", "0")) else f16
    vin = nc.dram_tensor("vin", [D, A + N], _fin, kind="ExternalInput")
    uin = nc.dram_tensor("uin", [D, A + NH], _fin, kind="ExternalInput")
    biasv = nc.dram_tensor("biasv", [128, 7 + 2 * FJ], f32, kind="ExternalInput")
    xkT = nc.dram_tensor("xkT", [N, D], f16, kind="ExternalInput")
    out = nc.dram_tensor("out", [NH, D], f32, kind="ExternalOutput")

    with tile.TileContext(nc) as tc:
        with (
            tc.tile_pool(name="consts", bufs=1) as consts,
            tc.tile_pool(name="ufeat", bufs=1) as ufeat,
            tc.tile_pool(name="uscal", bufs=1) as uscal,
            tc.tile_pool(name="tmpp", bufs=2) as tmpp,
            tc.tile_pool(name="gps", bufs=2, space="PSUM") as gps,
            tc.tile_pool(name="scps", bufs=1, space="PSUM") as scps,
            tc.tile_pool(name="attp", bufs=1) as attp,
            tc.tile_pool(name="opool", bufs=1) as opool,
        ):
            vin_sb = consts.tile([D, A + N], _fin, tag="vin")
            uin_sb = consts.tile([D, A + NH], _fin, tag="uin")
            biasv_sb = consts.tile([128, 7 + 2 * FJ], f32, tag="biasv")
            xkT_sb = consts.tile([128, 4, D], f16, tag="xkT")
            w2_sb = vin_sb[:, :A]
            xk_sb = vin_sb[:, A:A + N]
            w1_sb = uin_sb[:, :A]
            xq_sb = uin_sb[:, A:A + NH]
            bsin_sb = biasv_sb[:, 0:2]
            bcos_sb = biasv_sb[:, 2:4]
            wav_sb = biasv_sb[:, 4:6]
            sgb_sb = biasv_sb[:, 6:7]

            # dummy Sin on garbage to preload ACT table sets during DMAs
            dummy = consts.tile([128, 1], f32, tag="dummy")
            nc.gpsimd.memset(dummy[:], 0.0)
            nc.scalar.activation(dummy[:], dummy[:], AF.Sin)

            # split the critical vin transfer across both HWDGE queues
            nc.sync.dma_start(vin_sb[:, :A], vin.ap()[:, :A])
            nc.scalar.dma_start(vin_sb[:, A:], vin.ap()[:, A:])
            nc.sync.dma_start(biasv_sb[:], biasv.ap())
            nc.sync.dma_start(uin_sb[:], uin.ap())
            nc.scalar.dma_start(
                xkT_sb[:], xkT.ap().rearrange("(mb p) d -> p mb d", p=128)
            )

            # combined feature tiles, j = 1..FJ:
            # [128, (sin|cos), v-part(c*512+m) | u-part(1024 + c*256+n)]
            FV = N * 2            # 1024: v-part width
            FT = FV + NH * 2      # 1536: total width
            cf = [ufeat.tile([128, 2, FT], f16, tag=f"cf{j}", name=f"cf{j}")
                  if j >= 1 else None for j in range(FJ + 1)]
            us = [uscal.tile([128, 2, NH * 2], f16, tag=f"us{j}", name=f"us{j}")
                  if j >= 1 else None for j in range(FJ + 1)]
            twoc = consts.tile([128, 2, FT], f16, tag="twoc")

            # theta tiles + base features (j=1); v-side first, cos first
            HPI = float(np.pi / 2)
            if bg_zero:
                # bg == 0: immediate biases, chunk-merged theta tiles/sins
                thv = gps.tile([128, FV], f32, tag="thv", name="thv", bufs=1)
                for c in range(2):
                    nc.tensor.matmul(thv[:, c * N:(c + 1) * N],
                                     w2_sb[:, c * 128:(c + 1) * 128], xk_sb[:])
                thu = gps.tile([128, N], f32, tag="thu", name="thu", bufs=1)
                for c in range(2):
                    nc.tensor.matmul(thu[:, c * NH:(c + 1) * NH],
                                     w1_sb[:, c * 128:(c + 1) * 128], xq_sb[:])
                nc.scalar.activation(cf[1][:, 1, :FV], thv[:], AF.Sin,
                                     bias=bcos_sb[:, 0:1])
                nc.scalar.activation(cf[1][:, 1, FV:], thu[:], AF.Sin,
                                     bias=bcos_sb[:, 0:1])
                nc.scalar.activation(cf[1][:, 0, :FV], thv[:], AF.Sin,
                                     bias=bsin_sb[:, 0:1])
                nc.scalar.activation(cf[1][:, 0, FV:], thu[:], AF.Sin,
                                     bias=bsin_sb[:, 0:1])
            else:
                thvs = []
                for c in range(2):
                    thv = gps.tile([128, N], f32, tag="th", name=f"thv{c}")
                    nc.tensor.matmul(thv[:], w2_sb[:, c * 128:(c + 1) * 128],
                                     xk_sb[:])
                    thvs.append(thv)
                thus = []
                for c in range(2):
                    thu = gps.tile([128, N], f32, tag="th", name=f"thu{c}")
                    nc.tensor.matmul(thu[:, :NH],
                                     w1_sb[:, c * 128:(c + 1) * 128], xq_sb[:])
                    thus.append(thu)
                for c in range(2):
                    nc.scalar.activation(cf[1][:, 1, c * N:(c + 1) * N],
                                         thvs[c][:], AF.Sin,
                                         bias=bcos_sb[:, c:c + 1])
                    nc.scalar.activation(cf[1][:, 0, c * N:(c + 1) * N],
                                         thvs[c][:], AF.Sin,
                                         bias=bsin_sb[:, c:c + 1])
                for c in range(2):
                    nc.scalar.activation(
                        cf[1][:, 1, FV + c * NH:FV + (c + 1) * NH],
                        thus[c][:, :NH], AF.Sin, bias=bcos_sb[:, c:c + 1])
                    nc.scalar.activation(
                        cf[1][:, 0, FV + c * NH:FV + (c + 1) * NH],
                        thus[c][:, :NH], AF.Sin, bias=bsin_sb[:, c:c + 1])

            for fn in range(2):
                nc.vector.tensor_scalar_mul(twoc[:, fn, :], cf[1][:, 1, :], 2.0)

            sc = [scps.tile([128, NH], f32, tag=f"sc{mb}", name=f"sc{mb}")
                  for mb in range(4)]

            for j in range(1, FJ + 1):
                if j == 2:
                    # f_2 = 2c*f_1 - f_0 with f_0 = (0, 1):
                    # cos_2 first (only needs the cos lane), sin_2 direct
                    tmpc = tmpp.tile([128, 2, FT], f16, tag="tmpc")
                    nc.vector.tensor_mul(tmpc[:, 1, :], cf[1][:, 1, :],
                                         twoc[:, 1, :])
                    nc.vector.tensor_scalar_add(cf[2][:, 1, :], tmpc[:, 1, :],
                                                -1.0)
                    nc.vector.tensor_mul(cf[2][:, 0, :], cf[1][:, 0, :],
                                         twoc[:, 0, :])
                elif j == FJ:
                    # last harmonic: u-part first so the tail can start
                    tmpc = tmpp.tile([128, 2, FT], f16, tag="tmpc")
                    nc.vector.tensor_mul(tmpc[:], cf[j - 1][:], twoc[:])
                    nc.vector.tensor_sub(cf[j][:, :, FV:], tmpc[:, :, FV:],
                                         cf[j - 2][:, :, FV:])
                    nc.vector.tensor_sub(cf[j][:, :, :FV], tmpc[:, :, :FV],
                                         cf[j - 2][:, :, :FV])
                elif j >= 3:
                    tmpc = tmpp.tile([128, 2, FT], f16, tag="tmpc")
                    nc.vector.tensor_mul(tmpc[:], cf[j - 1][:], twoc[:])
                    nc.vector.tensor_sub(cf[j][:], tmpc[:], cf[j - 2][:])
                # scale u-part by Wa[a]*BJ[j-1] on the Scalar engine
                # (ACT Identity with per-partition scale; keeps DVE free)
                for c in range(2):
                    nc.scalar.activation(
                        us[j][:, :, c * NH:(c + 1) * NH],
                        cf[j][:, :, FV + c * NH:FV + (c + 1) * NH],
                        AF.Identity,
                        scale=biasv_sb[:, 7 + 2 * (j - 1) + c:
                                       8 + 2 * (j - 1) + c],
                    )
                # scoring: sin_u pairs cos_v, cos_u pairs sin_v
                for fn in range(2):
                    for c in range(2):
                        for mb in range(4):
                            nc.tensor.matmul(
                                sc[mb][:],
                                cf[j][:, 1 - fn,
                                      c * N + mb * 128: c * N + (mb + 1) * 128],
                                us[j][:, fn, c * NH:(c + 1) * NH],
                                start=(j == 1 and fn == 0 and c == 0),
                                stop=(j == FJ and fn == 1 and c == 1),
                                skip_group_check=True,
                            )

            attT = attp.tile([128, 4, NH], f16, tag="attT")
            out_sb = opool.tile([128, 2, D], f32, tag="out")
            # reuse the (dead) theta-tile PSUM slots for the final accums
            if bg_zero:
                fos = [gps.tile([128, D], f32, tag="thv", name="fo0", bufs=1),
                       gps.tile([128, D], f32, tag="thu", name="fo1", bufs=1)]
            else:
                fos = [gps.tile([128, D], f32, tag="th", name=f"fo{nb}")
                       for nb in range(2)]
            for mb in range(4):
                nc.scalar.activation(
                    attT[:, mb, :], sc[mb][:], AF.Sigmoid, bias=sgb_sb[:, 0:1]
                )
                for nb in range(2):
                    nc.tensor.matmul(
                        fos[nb][:],
                        attT[:, mb, nb * 128:(nb + 1) * 128],
                        xkT_sb[:, mb, :],
                        start=(mb == 0),
                        stop=(mb == 3),
                        skip_group_check=True,
                    )
            for nb in range(2):
                nc.vector.tensor_copy(out_sb[:, nb, :], fos[nb][:])

            nc.sync.dma_start(
                out.ap().rearrange("(nb p) d -> p nb d", p=128), out_sb[:]
            )

    nc.compile()
    return nc


def _prep_inputs_v2(x, Wg1, Wg2, bg, Wa_w, Wa_b, ba):
    """Host-side packing/slicing only (no reference math)."""
    x = np.asarray(x, np.float32)
    w1s = FS * np.asarray(Wg1, np.float32).T
    w2s = FS * np.asarray(Wg2, np.float32).T
    bgv = FS * np.asarray(bg, np.float32)
    biasv = np.empty((128, 7 + 2 * FJ), np.float32)
    biasv[:, 0:2] = bgv.reshape(2, 128).T
    biasv[:, 2:4] = bgv.reshape(2, 128).T + np.float32(np.pi / 2)
    biasv[:, 4:6] = np.asarray(Wa_w, np.float32).reshape(2, 128).T
    biasv[:, 6] = float(np.asarray(Wa_b).ravel()[0]) \
        + float(np.asarray(ba).ravel()[0])
    wac = np.asarray(Wa_w, np.float32).reshape(2, 128).T
    for j in range(1, FJ + 1):
        for c in range(2):
            biasv[:, 7 + 2 * (j - 1) + c] = wac[:, c] * np.float32(BJ[j - 1])
    in_maps = []
    for c in range(NCORES):
        b, half = c // 2, c % 2
        xb = x[b]
        import os
        dt = np.float32 if int(os.environ.get("K_F32IN", "0")) else np.float16
        vin = np.ascontiguousarray(np.concatenate([w2s, xb], axis=1), dtype=dt)
        uin = np.ascontiguousarray(
            np.concatenate([w1s, xb[:, half * NH:(half + 1) * NH]], axis=1),
            dtype=dt)
        in_maps.append({
            "vin": vin,
            "uin": uin,
            "biasv": np.ascontiguousarray(biasv),
            "xkT": np.ascontiguousarray(xb.T.astype(np.float16)),
        })
    return in_maps


def _run(inputs, trace=False):
    from concourse.bass_utils import run_bass_kernel_spmd

    bg_zero = bool(np.all(np.asarray(inputs["bg"]) == 0))
    key = ("nc", bg_zero)
    if key not in _cache:
        _cache[key] = _build_nc_v2(bg_zero=bg_zero)
    nc = _cache[key]
    in_maps = _prep_inputs_v2(**inputs)
    res = run_bass_kernel_spmd(
        nc, in_maps, core_ids=list(range(NCORES)), trace=trace
    )
    out = np.empty((B, N, D), np.float32)
    for c in range(NCORES):
        b, half = c // 2, c % 2
        out[b, half * NH:(half + 1) * NH] = res.results[c]["out"]
    return out, res


def kernel(**inputs):
    out, _ = _run(inputs, trace=False)
    return out
